# revision 1
# baseline (speedup 1.0000x reference)
"""GAT model (2-layer GAT + FC head) on 8 Trainium2 NeuronCores.

Strategy: destination-sharded. Each core owns 12544 (padded) dst nodes
= 98 windows of 128. Edges live on their dst's core, sorted into
(window, src-chunk) groups. Node phase computes per-node tables
[h | as] (bf16) sharded + AllGather; ad values stay core-local.
Edge phase: dma_gather of 512B records by src (int16 idx over 4
chunks of 25088 rows) + 256B ad rows by core-local dst; per-edge
softmax weights w = exp(leakyrelu(as+ad)) (no segment-max needed:
scores are bounded, exp cannot overflow in f32); messages
msg = w * [h | 1] scattered into per-window PSUM via one-hot matmuls
(one-hot built in bulk on DVE from iota==dstloc). Denominator rides
the matmul via the record's ones-column. FC head fused per window.
"""
import sys
import numpy as np
import ml_dtypes

sys.path.insert(0, "/opt/trn_rl_repo")

BF16 = ml_dtypes.bfloat16

N = 100000
E_RAW = 1600000
F_USER = 128
F_POST = 64
HID = 32
HEADS = 4
NEG = 0.2
CORES = 8
NPC = 12500                 # real nodes per core
NPC_PAD = 12544             # 98 * 128
WINDOWS = 98
N_PAD = NPC_PAD * CORES     # 100352
NCHUNK = 4
CHUNK = N_PAD // NCHUNK     # 25088
SW = 2                      # windows per superblock
D1 = 256                    # table1 row elems (bf16): [hblk 132 | as 4 | pad]
D2 = 128                    # table2 row elems: [h2blk 33 | as2 1 | pad]
DAD = 128                   # ad table row elems: [ad .. | pad]


def _g(v):
    """original node id -> padded global id"""
    return (v // NPC) * NPC_PAD + (v % NPC)


def _wrap_idx(flat):
    """flat int16 [G] -> wrapped+replicated [128, G//16]"""
    G = len(flat)
    w = flat.reshape(G // 16, 16).T  # [16, G/16]
    return np.tile(w, (8, 1)).copy()


def preprocess(edge_index):
    """Returns (static, per_core) where static describes the shared program
    shape and per_core[c] holds the input blobs."""
    src = np.asarray(edge_index[0], dtype=np.int64)
    dst = np.asarray(edge_index[1], dtype=np.int64)
    loops = np.arange(N, dtype=np.int64)
    src = np.concatenate([src, loops])
    dst = np.concatenate([dst, loops])
    sp = _g(src)
    dp = _g(dst)
    core = dst // NPC
    dloc_c = dst % NPC                      # 0..12499
    w = dloc_c // 128
    dloc_w = dloc_c % 128
    ch = sp // CHUNK
    srel = sp % CHUNK

    key = ((core * WINDOWS + w) * NCHUNK + ch).astype(np.int64)
    counts = np.bincount(key, minlength=CORES * WINDOWS * NCHUNK)
    counts = counts.reshape(CORES, WINDOWS, NCHUNK)
    maxc = counts.max(axis=0)               # [WINDOWS, NCHUNK]
    J = -(-maxc // 128)                     # ceil div; may be 0

    # superblocks
    sbs = [list(range(s, min(s + SW, WINDOWS))) for s in range(0, WINDOWS, SW)]

    # static slot layout per sb: chunk-major, then window
    sb_layout = []   # per sb: dict(ch -> [(w, slot_off_in_sb, J_w_ch)]), J_sb, per-window slot list
    for sb in sbs:
        off = 0
        per_ch = []
        win_slots = {ww: [] for ww in sb}
        for c in range(NCHUNK):
            groups = []
            for ww in sb:
                j = int(J[ww, c])
                if j == 0:
                    continue
                groups.append((ww, off, j))
                win_slots[ww].extend(range(off, off + j))
                off += j
            per_ch.append(groups)
        sb_layout.append(dict(per_ch=per_ch, J_sb=off, win_slots=win_slots))

    order = np.lexsort((srel, ch, w, core))
    so, wo, cho, srelo, dlwo, dlco = (
        x[order] for x in (sp, w, ch, srel, dloc_w, dloc_c))
    coreo = core[order]
    # group start offsets in sorted array per (core, w, ch)
    keyo = ((coreo * WINDOWS + wo) * NCHUNK + cho)
    starts = np.searchsorted(keyo, np.arange(CORES * WINDOWS * NCHUNK))
    ends = np.searchsorted(keyo, np.arange(CORES * WINDOWS * NCHUNK) + 1)

    per_core = []
    for c in range(CORES):
        src_blob = []
        ad_blob = []
        dl_blob = []
        for si, sb in enumerate(sbs):
            lay = sb_layout[si]
            J_sb = lay["J_sb"]
            ad_flat = np.zeros(J_sb * 128, np.int16)
            dl_arr = np.full((128, J_sb), 999.0, np.float32)
            for cidx in range(NCHUNK):
                groups = lay["per_ch"][cidx]
                if not groups:
                    continue
                G = 128 * sum(j for (_, _, j) in groups)
                idx_flat = np.zeros(G, np.int16)
                off0 = groups[0][1]
                for (ww, soff, j) in groups:
                    gi = (c * WINDOWS + ww) * NCHUNK + cidx
                    s0, s1 = int(starts[gi]), int(ends[gi])
                    n = s1 - s0
                    gbase = (soff - off0) * 128
                    idx_flat[gbase:gbase + n] = srelo[s0:s1].astype(np.int16)
                    ad_flat[soff * 128:soff * 128 + n] = dlco[s0:s1].astype(np.int16)
                    k = np.arange(n)
                    dl_arr[k % 128, soff + k // 128] = dlwo[s0:s1]
                src_blob.append(_wrap_idx(idx_flat).ravel())
            ad_blob.append(dl_arr.T.astype(BF16).ravel())  # dstlocT flat [J_sb*128]
            dl_blob.append(dl_arr.astype(BF16).ravel())
        per_core.append(dict(
            srcidx=np.concatenate(src_blob),
            dstloct=np.concatenate(ad_blob),
            dstloc=np.concatenate(dl_blob),
        ))
    static = dict(J=J, sbs=sbs, sb_layout=sb_layout)
    return static, per_core


def build_program(static, blob_sizes):
    import os
    mode = os.environ.get("KMODE", "full")
    import concourse.bass as bass
    import concourse.bacc as bacc
    import concourse.tile as tile
    from concourse import mybir

    F32, BF, I16 = mybir.dt.float32, mybir.dt.bfloat16, mybir.dt.int16
    AF = mybir.ActivationFunctionType
    OP = mybir.AluOpType
    sbs, lay = static["sbs"], static["sb_layout"]

    reps = int(os.environ.get("KREPS", "1"))
    nc = bacc.Bacc("TRN2", target_bir_lowering=False, debug=False)
    P = nc.declare_dram_parameter
    ut = P("ut", [128, NPC_PAD], BF, isOutput=False)
    postt = P("postt", [F_POST, NPC_PAD], BF, isOutput=False)
    w1a = P("w1a", [128, 140], BF, isOutput=False)
    w2a = P("w2a", [128, 35], BF, isOutput=False)
    fc1w = P("fc1w", [96, 32], BF, isOutput=False)
    fc2w = P("fc2w", [32, 1], BF, isOutput=False)
    fc1b = P("fc1b", [32, 1], F32, isOutput=False)
    fc2b = P("fc2b", [1, 1], F32, isOutput=False)
    b1rep = P("b1rep", [128, 128], F32, isOutput=False)
    b2rep = P("b2rep", [128, 32], F32, isOutput=False)
    iota = P("iota", [128, 128], BF, isOutput=False)
    identbf = P("identbf", [128, 128], BF, isOutput=False)
    identf = P("identf", [128, 128], F32, isOutput=False)
    ones4 = P("ones4", [128, 4], BF, isOutput=False)
    srcidx = P("srcidx", [blob_sizes["src"]], I16, isOutput=False)
    dstloct = P("dstloct", [blob_sizes["ad"]], BF, isOutput=False)
    iotacol = P("iotacol", [128, 1], F32, isOutput=False)
    dstloc = P("dstloc", [blob_sizes["dl"]], BF, isOutput=False)
    out_ext = P("out", [1, NPC_PAD], F32, isOutput=True)

    with tile.TileContext(nc) as tc:
        with (
            tc.tile_pool(name="cst", bufs=1) as cst,
            tc.tile_pool(name="sb", bufs=3) as sbp,
            tc.tile_pool(name="ps", bufs=2, space="PSUM") as psp,
            tc.tile_pool(name="dr", bufs=1, space="DRAM") as dr,
        ):
            tab1_shard = dr.tile([NPC_PAD, D1], BF)
            adtab1 = dr.tile([NPC_PAD, DAD], BF)
            tab2_shard = dr.tile([NPC_PAD, D2], BF)
            adtab2 = dr.tile([NPC_PAD, DAD], BF)
            x1t_dram = dr.tile([128, NPC_PAD], BF)

            iota_sb = cst.tile([128, 128], BF)
            identbf_sb = cst.tile([128, 128], BF)
            identf_sb = cst.tile([128, 128], F32)
            ones4_sb = cst.tile([128, 4], BF)
            iotacol_sb = cst.tile([128, 1], F32)
            w1a_sb = cst.tile([128, 140], BF)
            w2a_sb = cst.tile([128, 35], BF)
            fc1w_sb = cst.tile([96, 32], BF)
            fc2w_sb = cst.tile([32, 1], BF)
            fc1b_sb = cst.tile([32, 1], F32)
            fc2b_sb = cst.tile([1, 1], F32)
            b1rep_sb = cst.tile([128, 128], F32)
            b2rep_sb = cst.tile([128, 32], F32)
            for t, src in [(iota_sb, iota), (identbf_sb, identbf),
                           (identf_sb, identf), (ones4_sb, ones4), (iotacol_sb, iotacol),
                           (w1a_sb, w1a), (w2a_sb, w2a), (fc1w_sb, fc1w),
                           (fc2w_sb, fc2w), (fc1b_sb, fc1b), (fc2b_sb, fc2b),
                           (b1rep_sb, b1rep), (b2rep_sb, b2rep)]:
                nc.sync.dma_start(out=t[:], in_=src[:])

            for _rep in range(reps):
                tab1_full = dr.tile([N_PAD, D1], BF, addr_space="Shared",
                                    name=f"tab1_full_r{_rep}")
                tab2_full = dr.tile([N_PAD, D2], BF, addr_space="Shared",
                                    name=f"tab2_full_r{_rep}")
                # ---- node phase 1: tables for layer 1 ----
                for t in range(WINDOWS if mode != "min" else 0):
                    sl = slice(t * 128, (t + 1) * 128)
                    lh = sbp.tile([128, 128], BF, tag="lh")
                    nc.sync.dma_start(out=lh[:], in_=ut[:, sl])
                    acc = psp.tile([128, 140], F32, tag="acc", space="PSUM")
                    nc.tensor.matmul(out=acc[:], lhsT=lh[:], rhs=w1a_sb[:],
                                     start=True, stop=True)
                    rec = sbp.tile([128, D1], BF, tag="nrec")
                    nc.vector.tensor_copy(out=rec[:, 0:136], in_=acc[:, 0:136])
                    nc.vector.tensor_copy(
                        out=rec[:, 0:132].rearrange("p (h f) -> p h f", f=33)[:, :, 32],
                        in_=ones4_sb[:])
                    nc.sync.dma_start(out=tab1_shard[sl, :], in_=rec[:])
                    ad4 = sbp.tile([128, 4], BF, tag="ad4")
                    nc.vector.tensor_copy(out=ad4[:], in_=acc[:, 136:140])
                    nc.sync.dma_start(out=adtab1[sl, 0:4], in_=ad4[:])

                if mode not in ("noag", "min"):
                    nc.gpsimd.collective_compute(
                        "AllGather", mybir.AluOpType.bypass,
                        ins=[tab1_shard[:].opt()], outs=[tab1_full[:].opt()],
                        replica_groups=[list(range(CORES))])

                # ---- generic edge phase ----
                def edge_phase(tabfull, adtab, elem, H, mcols, epilogue, blob_offs):
                    so, ao, do = blob_offs
                    for si, sb in enumerate(sbs):
                        layd = lay[si]
                        J_sb = layd["J_sb"]
                        rec = sbp.tile([128, J_sb * elem], BF, tag="erec", bufs=2)
                        for cidx in range(NCHUNK):
                            groups = layd["per_ch"][cidx]
                            if not groups:
                                continue
                            Jch = sum(j for (_, _, j) in groups)
                            off0 = groups[0][1]
                            G = 128 * Jch
                            idxt = sbp.tile([128, G // 16], I16, tag=f"idx{cidx}")
                            nc.sync.dma_start(
                                out=idxt[:],
                                in_=srcidx[so:so + 128 * (G // 16)].rearrange(
                                    "(p s) -> p s", s=G // 16))
                            so += 128 * (G // 16)
                            if mode in ("nogather",):
                                continue
                            nc.gpsimd.dma_gather(
                                out_ap=rec[:, off0 * elem:(off0 + Jch) * elem]
                                    .rearrange("p (j d) -> p j d", d=elem),
                                in_ap=tabfull[cidx * CHUNK:(cidx + 1) * CHUNK, :],
                                idxs_ap=idxt[:], num_idxs=G, num_idxs_reg=G,
                                elem_size=elem, single_packet=False)
                        Gad = J_sb * 128
                        dtr = sbp.tile([128, Gad], BF, tag="adE", bufs=2)
                        nc.sync.dma_start(
                            out=dtr[:],
                            in_=dstloct[ao:ao + Gad][None, :].to_broadcast([128, Gad]))
                        ao += Gad
                        ohT = sbp.tile([128, Gad], BF, tag="ohT", bufs=2)
                        nc.vector.tensor_scalar(
                            out=ohT[:], in0=dtr[:], scalar1=iotacol_sb[:, 0:1],
                            scalar2=None, op0=OP.is_equal)
                        adp = psp.tile([128, J_sb * H], F32, tag="adp", space="PSUM")
                        for ww2 in sb:
                            adw = sbp.tile([128, H], BF, tag="adw")
                            nc.sync.dma_start(
                                out=adw[:], in_=adtab[ww2 * 128:(ww2 + 1) * 128, 0:H])
                            for s_ in layd["win_slots"][ww2]:
                                nc.tensor.matmul(
                                    out=adp[:, s_ * H:(s_ + 1) * H],
                                    lhsT=ohT[:, s_ * 128:(s_ + 1) * 128],
                                    rhs=adw[:], start=True, stop=True)
                        dl = sbp.tile([128, J_sb], BF, tag="dl")
                        nc.sync.dma_start(
                            out=dl[:],
                            in_=dstloc[do:do + 128 * J_sb].rearrange(
                                "(p s) -> p s", s=J_sb))
                        do += 128 * J_sb

                        if mode == "nocompute":
                            continue
                        recv = rec[:].rearrange("p (j d) -> p j d", d=elem)
                        adc = sbp.tile([128, J_sb * H], BF, tag="adc")
                        nc.vector.tensor_copy(out=adc[:], in_=adp[:])
                        e1 = sbp.tile([128, J_sb * H], F32, tag="e1")
                        nc.vector.tensor_tensor(
                            out=e1[:].rearrange("p (j h) -> p j h", h=H),
                            in0=recv[:, :, mcols:mcols + H],
                            in1=adc[:].rearrange("p (j h) -> p j h", h=H),
                            op=OP.add)
                        lr = sbp.tile([128, J_sb * H], F32, tag="lr")
                        nc.vector.tensor_scalar_mul(out=lr[:], in0=e1[:], scalar1=NEG)
                        nc.vector.tensor_tensor(out=e1[:], in0=e1[:], in1=lr[:], op=OP.max)
                        wgt = sbp.tile([128, J_sb * H], BF, tag="wgt")
                        nc.scalar.activation(out=wgt[:], in_=e1[:], func=AF.Exp)
                        msg = sbp.tile([128, J_sb * mcols], BF, tag="msg", bufs=2)
                        nc.vector.tensor_tensor(
                            out=msg[:].rearrange("p (j h f) -> p j h f", h=H, f=mcols // H),
                            in0=recv[:, :, 0:mcols].rearrange(
                                "p j (h f) -> p j h f", f=mcols // H),
                            in1=wgt[:].rearrange("p (j h) -> p j h", h=H)[:, :, :, None]
                                .to_broadcast([128, J_sb, H, mcols // H]),
                            op=OP.mult)
                        oh = sbp.tile([128, J_sb * 128], BF, tag="oh", bufs=2)
                        nc.vector.tensor_tensor(
                            out=oh[:].rearrange("p (j f) -> p j f", f=128),
                            in0=iota_sb[:][:, None, :].to_broadcast([128, J_sb, 128]),
                            in1=dl[:][:, :, None].to_broadcast([128, J_sb, 128]),
                            op=OP.is_equal)
                        for ww in sb:
                            slots = layd["win_slots"][ww]
                            if not slots:
                                continue
                            acc = psp.tile([128, mcols], F32, tag="acc", space="PSUM")
                            for i, s in enumerate(slots):
                                nc.tensor.matmul(
                                    out=acc[:],
                                    lhsT=oh[:, s * 128:(s + 1) * 128],
                                    rhs=msg[:, s * mcols:(s + 1) * mcols],
                                    start=(i == 0), stop=(i == len(slots) - 1))
                            epilogue(ww, acc)

                # ---- layer 1 epilogue ----
                def epi1(ww, acc):
                    den = sbp.tile([128, 4], F32, tag="den")
                    nc.vector.tensor_copy(
                        out=den[:],
                        in_=acc[:].rearrange("p (h f) -> p h f", f=33)[:, :, 32])
                    nc.vector.tensor_scalar_max(out=den[:], in0=den[:], scalar1=1e-30)
                    rcp = sbp.tile([128, 4], F32, tag="rcp")
                    nc.vector.reciprocal(out=rcp[:], in_=den[:])
                    x1 = sbp.tile([128, 128], F32, tag="x1")
                    accv = acc[:].rearrange("p (h f) -> p h f", f=33)
                    for h in range(HEADS):
                        nc.vector.tensor_scalar(
                            out=x1[:, h * 32:(h + 1) * 32],
                            in0=accv[:, h, 0:32],
                            scalar1=rcp[:, h:h + 1], scalar2=None, op0=OP.mult)
                    nc.vector.tensor_tensor(out=x1[:], in0=x1[:], in1=b1rep_sb[:], op=OP.add)
                    x1b = sbp.tile([128, 128], BF, tag="x1b")
                    nc.scalar.activation(out=x1b[:], in_=x1[:], func=AF.Relu)
                    tp = psp.tile([128, 128], BF, tag="tp", space="PSUM")
                    nc.tensor.transpose(out=tp[:], in_=x1b[:], identity=identbf_sb[:])
                    x1t = sbp.tile([128, 128], BF, tag="x1t")
                    nc.vector.tensor_copy(out=x1t[:], in_=tp[:])
                    nc.sync.dma_start(
                        out=x1t_dram[:, ww * 128:(ww + 1) * 128], in_=x1t[:])

                if mode not in ("noedge", "noag", "min"):
                    edge_phase(tab1_full, adtab1, D1, HEADS, 132, epi1, (0, 0, 0))

                # ---- node phase 2 ----
                for t in range(WINDOWS if mode != "min" else 0):
                    sl = slice(t * 128, (t + 1) * 128)
                    lh2 = sbp.tile([128, 128], BF, tag="lh")
                    nc.sync.dma_start(out=lh2[:], in_=x1t_dram[:, sl])
                    acc = psp.tile([128, 35], F32, tag="acc", space="PSUM")
                    nc.tensor.matmul(out=acc[:], lhsT=lh2[:], rhs=w2a_sb[:],
                                     start=True, stop=True)
                    rec2 = sbp.tile([128, D2], BF, tag="nrec")
                    nc.vector.tensor_copy(out=rec2[:, 0:34], in_=acc[:, 0:34])
                    nc.vector.tensor_copy(out=rec2[:, 32:33], in_=ones4_sb[:, 0:1])
                    nc.sync.dma_start(out=tab2_shard[sl, :], in_=rec2[:])
                    ad1c = sbp.tile([128, 1], BF, tag="ad4")
                    nc.vector.tensor_copy(out=ad1c[:], in_=acc[:, 34:35])
                    nc.sync.dma_start(out=adtab2[sl, 0:1], in_=ad1c[:])

                if mode not in ("noag", "min"):
                    nc.gpsimd.collective_compute(
                        "AllGather", mybir.AluOpType.bypass,
                        ins=[tab2_shard[:].opt()], outs=[tab2_full[:].opt()],
                        replica_groups=[list(range(CORES))])

                # ---- layer 2 epilogue (+ fused FC head) ----
                def epi2(ww, acc):
                    den = sbp.tile([128, 1], F32, tag="den")
                    nc.vector.tensor_copy(out=den[:], in_=acc[:, 32:33])
                    nc.vector.tensor_scalar_max(out=den[:], in0=den[:], scalar1=1e-30)
                    rcp = sbp.tile([128, 1], F32, tag="rcp")
                    nc.vector.reciprocal(out=rcp[:], in_=den[:])
                    x2 = sbp.tile([128, 32], F32, tag="x2")
                    nc.vector.tensor_scalar(
                        out=x2[:], in0=acc[:, 0:32],
                        scalar1=rcp[:, 0:1], scalar2=None, op0=OP.mult)
                    nc.vector.tensor_tensor(out=x2[:], in0=x2[:], in1=b2rep_sb[:], op=OP.add)
                    x2f = sbp.tile([128, 32], F32, tag="x2f")
                    nc.scalar.activation(out=x2f[:], in_=x2[:], func=AF.Relu)
                    tp2 = psp.tile([32, 128], F32, tag="tp", space="PSUM")
                    nc.tensor.transpose(out=tp2[:], in_=x2f[:], identity=identf_sb[:])
                    zt = sbp.tile([96, 128], BF, tag="zt")
                    nc.vector.tensor_copy(out=zt[0:32, :], in_=tp2[:])
                    nc.sync.dma_start(out=zt[32:96, :],
                                      in_=postt[:, ww * 128:(ww + 1) * 128])
                    pa = psp.tile([32, 128], F32, tag="fc", space="PSUM")
                    nc.tensor.matmul(out=pa[:], lhsT=fc1w_sb[:], rhs=zt[:],
                                     start=True, stop=True)
                    y1 = sbp.tile([32, 128], BF, tag="y1")
                    nc.scalar.activation(out=y1[:], in_=pa[:], func=AF.Relu,
                                         bias=fc1b_sb[:])
                    pb = psp.tile([1, 128], F32, tag="fc", space="PSUM")
                    nc.tensor.matmul(out=pb[:], lhsT=fc2w_sb[:], rhs=y1[:],
                                     start=True, stop=True)
                    yo = sbp.tile([1, 128], F32, tag="yo")
                    nc.scalar.activation(out=yo[:], in_=pb[:], func=AF.Sigmoid,
                                         bias=fc2b_sb[:])
                    nc.sync.dma_start(out=out_ext[0:1, ww * 128:(ww + 1) * 128],
                                      in_=yo[:])

                if mode not in ("noedge", "noag", "min"):
                    edge_phase(tab2_full, adtab2, D2, 1, 33, epi2, (0, 0, 0))
            if mode == "min":
                zo = sbp.tile([1, NPC_PAD], F32, tag="zo")
                nc.vector.memset(zo[:], 0.5)
                nc.sync.dma_start(out=out_ext[:], in_=zo[:])

    nc.compile()
    return nc


def _make_inputs(user_features, post_features, W1, a1s, a1d, b1,
                 W2, a2s, a2d, b2, fc1_w, fc1_b, fc2_w, fc2_b, per_core):
    uf = np.asarray(user_features, np.float32)
    pf = np.asarray(post_features, np.float32)
    W1 = np.asarray(W1, np.float32)
    W2 = np.asarray(W2, np.float32)
    a1s = np.asarray(a1s, np.float32)
    a1d = np.asarray(a1d, np.float32)
    a2s = np.asarray(a2s, np.float32)
    a2d = np.asarray(a2d, np.float32)

    w1a = np.zeros((128, 140), np.float32)
    for h in range(HEADS):
        w1a[:, h * 33:h * 33 + 32] = W1[:, h * 32:(h + 1) * 32]
        w1a[:, 132 + h] = W1[:, h * 32:(h + 1) * 32] @ a1s[h]
        w1a[:, 136 + h] = W1[:, h * 32:(h + 1) * 32] @ a1d[h]
    w2a = np.zeros((128, 35), np.float32)
    w2a[:, 0:32] = W2
    w2a[:, 33] = W2 @ a2s[0]
    w2a[:, 34] = W2 @ a2d[0]

    iota = np.tile(np.arange(128, dtype=np.float32), (128, 1))
    base = dict(
        w1a=w1a.astype(BF16), w2a=w2a.astype(BF16),
        fc1w=np.asarray(fc1_w, np.float32).astype(BF16),
        fc2w=np.asarray(fc2_w, np.float32).astype(BF16),
        fc1b=np.asarray(fc1_b, np.float32).reshape(32, 1).copy(),
        fc2b=np.asarray(fc2_b, np.float32).reshape(1, 1).copy(),
        b1rep=np.tile(np.asarray(b1, np.float32), (128, 1)),
        b2rep=np.tile(np.asarray(b2, np.float32), (128, 1)),
        iota=iota.astype(BF16),
        identbf=np.eye(128, dtype=np.float32).astype(BF16),
        identf=np.eye(128, dtype=np.float32),
        ones4=np.ones((128, 4), np.float32).astype(BF16),
        iotacol=np.arange(128, dtype=np.float32).reshape(128, 1),
    )
    in_maps = []
    for c in range(CORES):
        sl = slice(c * NPC, (c + 1) * NPC)
        ut = np.zeros((128, NPC_PAD), np.float32)
        ut[:, :NPC] = uf[sl].T
        postt = np.zeros((F_POST, NPC_PAD), np.float32)
        postt[:, :NPC] = pf[sl].T
        m = dict(base)
        m["ut"] = ut.astype(BF16)
        m["postt"] = postt.astype(BF16)
        m.update(per_core[c])
        in_maps.append(m)
    return in_maps


_CACHE = {}
LAST_EXEC_NS = None


def kernel(**inputs):
    from concourse.bass_utils import run_bass_kernel_spmd
    ei = np.asarray(inputs["edge_index"])
    static, per_core = preprocess(ei)
    blob_sizes = dict(src=len(per_core[0]["srcidx"]),
                      ad=len(per_core[0]["dstloct"]),
                      dl=len(per_core[0]["dstloc"]))
    in_maps = _make_inputs(
        inputs["user_features"], inputs["post_features"],
        inputs["W1"], inputs["a1s"], inputs["a1d"], inputs["b1"],
        inputs["W2"], inputs["a2s"], inputs["a2d"], inputs["b2"],
        inputs["fc1_w"], inputs["fc1_b"], inputs["fc2_w"], inputs["fc2_b"],
        per_core)
    key = (blob_sizes["src"], blob_sizes["ad"], blob_sizes["dl"])
    if key not in _CACHE:
        _CACHE[key] = build_program(static, blob_sizes)
    nc = _CACHE[key]
    import os
    trace = bool(os.environ.get("BASS_KERNEL_TRACE"))
    r = run_bass_kernel_spmd(nc, in_maps, list(range(CORES)), trace=trace)
    global LAST_EXEC_NS
    LAST_EXEC_NS = r.exec_time_ns
    out = np.empty((N, 1), np.float32)
    for c in range(CORES):
        out[c * NPC:(c + 1) * NPC, 0] = r.results[c]["out"][0, :NPC]
    return out



# revision 9
# speedup vs baseline: 1.9661x; 1.9661x over previous
"""GAT model (2-layer GAT + FC head) on 8 Trainium2 NeuronCores.

Strategy: destination-sharded. Each core owns 12544 (padded) dst nodes
= 98 windows of 128. Edges live on their dst's core, sorted into
(window, src-chunk) groups. Node phase computes per-node tables
[h | as] (bf16) sharded + AllGather; ad values stay core-local.
Edge phase: dma_gather of 512B records by src (int16 idx over 4
chunks of 25088 rows) + broadcast of dst-local ids; per-edge
softmax weights w = exp(leakyrelu(as+ad)) (no segment-max needed:
scores are bounded, exp cannot overflow in f32); messages
msg = w * [h | 1] scattered into per-window PSUM via one-hot matmuls
(one-hot built in bulk on DVE from iota==dstloc). Denominator rides
the matmul via the record's ones-column. FC head fused per window.

Host->device upload is the wall-clock bottleneck (axon PJRT tunnel),
so inputs are minimized: gather indices uploaded unreplicated
[16, cols] and tiled 8x across partitions on device into one
resident SBUF tile; dst-local ids as uint8 (cast on device);
node/post features as scaled int8 (cast to bf16 on device,
scales folded into w1a/fc1w on host); iota /
identity / replicated-bias constants built on device.
"""
import sys
import numpy as np
import ml_dtypes

sys.path.insert(0, "/opt/trn_rl_repo")

BF16 = ml_dtypes.bfloat16

N = 100000
E_RAW = 1600000
F_USER = 128
F_POST = 64
HID = 32
HEADS = 4
NEG = 0.2
CORES = 8
NPC = 12500                 # real nodes per core
NPC_PAD = 12544             # 98 * 128
WINDOWS = 98
N_PAD = NPC_PAD * CORES     # 100352
NCHUNK = 4
CHUNK = N_PAD // NCHUNK     # 25088
SW = 2                      # windows per superblock
D1 = 256                    # table1 row elems (bf16): [hblk 132 | as 4 | pad]
D2 = 128                    # table2 row elems: [h2blk 33 | as2 1 | pad]
DAD = 128                   # ad table row elems: [ad .. | pad]
UT_I8 = True                # upload node/post features as scaled int8


def _g(v):
    """original node id -> padded global id"""
    return (v // NPC) * NPC_PAD + (v % NPC)


def preprocess(edge_index):
    """Returns (static, per_core) where static describes the shared program
    shape and per_core[c] holds the input blobs."""
    src = np.asarray(edge_index[0], dtype=np.int64)
    dst = np.asarray(edge_index[1], dtype=np.int64)
    loops = np.arange(N, dtype=np.int64)
    src = np.concatenate([src, loops])
    dst = np.concatenate([dst, loops])
    sp = _g(src)
    core = dst // NPC
    dloc_c = dst % NPC                      # 0..12499
    w = dloc_c // 128
    dloc_w = dloc_c % 128
    ch = sp // CHUNK
    srel = sp % CHUNK

    key = ((core * WINDOWS + w) * NCHUNK + ch).astype(np.int64)
    counts = np.bincount(key, minlength=CORES * WINDOWS * NCHUNK)
    counts = counts.reshape(CORES, WINDOWS, NCHUNK)
    maxc = counts.max(axis=0)               # [WINDOWS, NCHUNK]
    J = -(-maxc // 128)                     # ceil div; may be 0

    # superblocks
    sbs = [list(range(s, min(s + SW, WINDOWS))) for s in range(0, WINDOWS, SW)]

    # static slot layout per sb: chunk-major, then window
    sb_layout = []   # per sb: dict(ch -> [(w, slot_off_in_sb, J_w_ch)]), J_sb, per-window slot list
    for sb in sbs:
        off = 0
        per_ch = []
        win_slots = {ww: [] for ww in sb}
        for c in range(NCHUNK):
            groups = []
            for ww in sb:
                j = int(J[ww, c])
                if j == 0:
                    continue
                groups.append((ww, off, j))
                win_slots[ww].extend(range(off, off + j))
                off += j
            per_ch.append(groups)
        sb_layout.append(dict(per_ch=per_ch, J_sb=off, win_slots=win_slots))

    # static column offsets of each (sb, chunk) block in the resident idx tile
    idx_colo = []
    tot_cols = 0
    for si, sb in enumerate(sbs):
        cc = []
        for cidx in range(NCHUNK):
            groups = sb_layout[si]["per_ch"][cidx]
            Jch = sum(j for (_, _, j) in groups)
            cc.append(tot_cols)
            tot_cols += 8 * Jch             # (128*Jch)/16 columns
        idx_colo.append(cc)

    order = np.lexsort((srel, ch, w, core))
    so, wo, cho, srelo, dlwo, dlco = (
        x[order] for x in (sp, w, ch, srel, dloc_w, dloc_c))
    coreo = core[order]
    # group start offsets in sorted array per (core, w, ch)
    keyo = ((coreo * WINDOWS + wo) * NCHUNK + cho)
    starts = np.searchsorted(keyo, np.arange(CORES * WINDOWS * NCHUNK))
    ends = np.searchsorted(keyo, np.arange(CORES * WINDOWS * NCHUNK) + 1)

    per_core = []
    for c in range(CORES):
        idx_cols = []      # [16, cols] blocks, horizontally concatenated
        ad_blob = []
        dl_blob = []
        for si, sb in enumerate(sbs):
            lay = sb_layout[si]
            J_sb = lay["J_sb"]
            dl_arr = np.full((128, J_sb), 255, np.uint8)
            for cidx in range(NCHUNK):
                groups = lay["per_ch"][cidx]
                if not groups:
                    continue
                G = 128 * sum(j for (_, _, j) in groups)
                idx_flat = np.zeros(G, np.int16)
                off0 = groups[0][1]
                for (ww, soff, j) in groups:
                    gi = (c * WINDOWS + ww) * NCHUNK + cidx
                    s0, s1 = int(starts[gi]), int(ends[gi])
                    n = s1 - s0
                    gbase = (soff - off0) * 128
                    idx_flat[gbase:gbase + n] = srelo[s0:s1].astype(np.int16)
                    k = np.arange(n)
                    dl_arr[k % 128, soff + k // 128] = dlwo[s0:s1]
                idx_cols.append(idx_flat.reshape(G // 16, 16).T)  # [16, G/16]
            ad_blob.append(dl_arr.T.ravel())   # dstlocT flat [J_sb*128] u8
            dl_blob.append(dl_arr.ravel())
        per_core.append(dict(
            srcidx=np.ascontiguousarray(np.concatenate(idx_cols, axis=1)),
            dstloct=np.concatenate(ad_blob),
            dstloc=np.concatenate(dl_blob),
        ))
    static = dict(J=J, sbs=sbs, sb_layout=sb_layout, idx_colo=idx_colo,
                  tot_cols=tot_cols)
    return static, per_core


def build_program(static, blob_sizes):
    import os
    mode = os.environ.get("KMODE", "full")
    import concourse.bass as bass
    import concourse.bacc as bacc
    import concourse.tile as tile
    from concourse import mybir

    F32, BF, I16 = mybir.dt.float32, mybir.dt.bfloat16, mybir.dt.int16
    U8, I32, I8 = mybir.dt.uint8, mybir.dt.int32, mybir.dt.int8
    FEAT = I8 if UT_I8 else BF
    AF = mybir.ActivationFunctionType
    OP = mybir.AluOpType
    sbs, lay = static["sbs"], static["sb_layout"]
    idx_colo, TOT_COLS = static["idx_colo"], static["tot_cols"]

    reps = int(os.environ.get("KREPS", "1"))
    nc = bacc.Bacc("TRN2", target_bir_lowering=False, debug=False)
    P = nc.declare_dram_parameter
    ut = P("ut", [128, NPC_PAD], FEAT, isOutput=False)
    postt = P("postt", [F_POST, NPC_PAD], FEAT, isOutput=False)
    w1a = P("w1a", [128, 140], BF, isOutput=False)
    w2a = P("w2a", [128, 35], BF, isOutput=False)
    fc1w = P("fc1w", [96, 32], BF, isOutput=False)
    fc2w = P("fc2w", [32, 1], BF, isOutput=False)
    fc1b = P("fc1b", [32, 1], F32, isOutput=False)
    fc2b = P("fc2b", [1, 1], F32, isOutput=False)
    b1row = P("b1row", [1, 128], F32, isOutput=False)
    b2row = P("b2row", [1, 32], F32, isOutput=False)
    srcidx = P("srcidx", [16, TOT_COLS], I16, isOutput=False)
    dstloct = P("dstloct", [blob_sizes["ad"]], U8, isOutput=False)
    dstloc = P("dstloc", [blob_sizes["dl"]], U8, isOutput=False)
    out_ext = P("out", [1, NPC_PAD], F32, isOutput=True)

    with tile.TileContext(nc) as tc:
        with (
            tc.tile_pool(name="cst", bufs=1) as cst,
            tc.tile_pool(name="sb", bufs=3) as sbp,
            tc.tile_pool(name="ps", bufs=2, space="PSUM") as psp,
            tc.tile_pool(name="dr", bufs=1, space="DRAM") as dr,
        ):
            tab1_shard = dr.tile([NPC_PAD, D1], BF)
            adtab1 = dr.tile([NPC_PAD, DAD], BF)
            tab2_shard = dr.tile([NPC_PAD, D2], BF)
            adtab2 = dr.tile([NPC_PAD, DAD], BF)
            x1t_dram = dr.tile([128, NPC_PAD], BF)

            w1a_sb = cst.tile([128, 140], BF)
            w2a_sb = cst.tile([128, 35], BF)
            fc1w_sb = cst.tile([96, 32], BF)
            fc2w_sb = cst.tile([32, 1], BF)
            fc1b_sb = cst.tile([32, 1], F32)
            fc2b_sb = cst.tile([1, 1], F32)
            b1rep_sb = cst.tile([128, 128], F32)
            b2rep_sb = cst.tile([128, 32], F32)
            for t, src in [(w1a_sb, w1a), (w2a_sb, w2a), (fc1w_sb, fc1w),
                           (fc2w_sb, fc2w), (fc1b_sb, fc1b), (fc2b_sb, fc2b)]:
                nc.sync.dma_start(out=t[:], in_=src[:])
            nc.sync.dma_start(out=b1rep_sb[:],
                              in_=b1row[0:1, :].to_broadcast([128, 128]))
            nc.sync.dma_start(out=b2rep_sb[:],
                              in_=b2row[0:1, :].to_broadcast([128, 32]))

            # on-device constants: iota row/col, identities, ones
            iotar_i = cst.tile([128, 128], I16)
            nc.gpsimd.iota(iotar_i[:], pattern=[[1, 128]], channel_multiplier=0)
            iota_sb = cst.tile([128, 128], BF)
            nc.vector.tensor_copy(out=iota_sb[:], in_=iotar_i[:])
            iotac_i = cst.tile([128, 1], I32)
            nc.gpsimd.iota(iotac_i[:], pattern=[[0, 1]], channel_multiplier=1)
            iotacol_sb = cst.tile([128, 1], F32)
            nc.vector.tensor_copy(out=iotacol_sb[:], in_=iotac_i[:])
            identbf_sb = cst.tile([128, 128], BF)
            nc.vector.tensor_scalar(
                out=identbf_sb[:], in0=iota_sb[:], scalar1=iotacol_sb[:, 0:1],
                scalar2=None, op0=OP.is_equal)
            identf_sb = cst.tile([128, 128], F32)
            nc.vector.tensor_scalar(
                out=identf_sb[:], in0=iota_sb[:], scalar1=iotacol_sb[:, 0:1],
                scalar2=None, op0=OP.is_equal)
            ones4_sb = cst.tile([128, 4], BF)
            nc.vector.memset(ones4_sb[:], 1.0)

            for _rep in range(reps):
                tab1_full = dr.tile([N_PAD, D1], BF, addr_space="Shared",
                                    name=f"tab1_full_r{_rep}")
                tab2_full = dr.tile([N_PAD, D2], BF, addr_space="Shared",
                                    name=f"tab2_full_r{_rep}")
                # ---- node phase 1: tables for layer 1 ----
                for t in range(WINDOWS if mode != "min" else 0):
                    sl = slice(t * 128, (t + 1) * 128)
                    if UT_I8:
                        lh8 = sbp.tile([128, 128], I8, tag="lh8")
                        nc.sync.dma_start(out=lh8[:], in_=ut[:, sl])
                        lh = sbp.tile([128, 128], BF, tag="lh")
                        nc.vector.tensor_copy(out=lh[:], in_=lh8[:])
                    else:
                        lh = sbp.tile([128, 128], BF, tag="lh")
                        nc.sync.dma_start(out=lh[:], in_=ut[:, sl])
                    acc = psp.tile([128, 140], F32, tag="acc", space="PSUM")
                    nc.tensor.matmul(out=acc[:], lhsT=lh[:], rhs=w1a_sb[:],
                                     start=True, stop=True)
                    rec = sbp.tile([128, D1], BF, tag="nrec")
                    nc.vector.tensor_copy(out=rec[:, 0:136], in_=acc[:, 0:136])
                    nc.vector.tensor_copy(
                        out=rec[:, 0:132].rearrange("p (h f) -> p h f", f=33)[:, :, 32],
                        in_=ones4_sb[:])
                    nc.sync.dma_start(out=tab1_shard[sl, :], in_=rec[:])
                    ad4 = sbp.tile([128, 4], BF, tag="ad4")
                    nc.vector.tensor_copy(out=ad4[:], in_=acc[:, 136:140])
                    nc.sync.dma_start(out=adtab1[sl, 0:4], in_=ad4[:])

                if mode not in ("noag", "min"):
                    nc.gpsimd.collective_compute(
                        "AllGather", mybir.AluOpType.bypass,
                        ins=[tab1_shard[:].opt()], outs=[tab1_full[:].opt()],
                        replica_groups=[list(range(CORES))])

                # ---- generic edge phase ----
                def edge_phase(tabfull, adtab, elem, H, mcols, epilogue, blob_offs):
                    ao, do = blob_offs
                    for si, sb in enumerate(sbs):
                        layd = lay[si]
                        J_sb = layd["J_sb"]
                        c0 = idx_colo[si][0]
                        c1 = (idx_colo[si + 1][0] if si + 1 < len(sbs)
                              else TOT_COLS)
                        sbcols = c1 - c0
                        idxt = sbp.tile([128, sbcols], I16, tag="idxt", bufs=2)
                        for r in range(8):
                            nc.sync.dma_start(
                                out=idxt[16 * r:16 * r + 16, :],
                                in_=srcidx[:, c0:c1])
                        rec = sbp.tile([128, J_sb * elem], BF, tag="erec", bufs=2)
                        for cidx in range(NCHUNK):
                            groups = layd["per_ch"][cidx]
                            if not groups:
                                continue
                            Jch = sum(j for (_, _, j) in groups)
                            off0 = groups[0][1]
                            G = 128 * Jch
                            if mode in ("nogather",):
                                continue
                            nc.gpsimd.dma_gather(
                                out_ap=rec[:, off0 * elem:(off0 + Jch) * elem]
                                    .rearrange("p (j d) -> p j d", d=elem),
                                in_ap=tabfull[cidx * CHUNK:(cidx + 1) * CHUNK, :],
                                idxs_ap=idxt[:, idx_colo[si][cidx] - c0:
                                             idx_colo[si][cidx] - c0 + 8 * Jch],
                                num_idxs=G, num_idxs_reg=G,
                                elem_size=elem, single_packet=False)
                        Gad = J_sb * 128
                        dtr8 = sbp.tile([128, Gad], U8, tag="adE8", bufs=2)
                        nc.sync.dma_start(
                            out=dtr8[:],
                            in_=dstloct[ao:ao + Gad][None, :].to_broadcast([128, Gad]))
                        ao += Gad
                        dtr = sbp.tile([128, Gad], BF, tag="adE", bufs=2)
                        nc.vector.tensor_copy(out=dtr[:], in_=dtr8[:])
                        ohT = sbp.tile([128, Gad], BF, tag="ohT", bufs=2)
                        nc.vector.tensor_scalar(
                            out=ohT[:], in0=dtr[:], scalar1=iotacol_sb[:, 0:1],
                            scalar2=None, op0=OP.is_equal)
                        adp = psp.tile([128, J_sb * H], F32, tag="adp", space="PSUM")
                        for ww2 in sb:
                            adw = sbp.tile([128, H], BF, tag="adw")
                            nc.sync.dma_start(
                                out=adw[:], in_=adtab[ww2 * 128:(ww2 + 1) * 128, 0:H])
                            for s_ in layd["win_slots"][ww2]:
                                nc.tensor.matmul(
                                    out=adp[:, s_ * H:(s_ + 1) * H],
                                    lhsT=ohT[:, s_ * 128:(s_ + 1) * 128],
                                    rhs=adw[:], start=True, stop=True)
                        dl8 = sbp.tile([128, J_sb], U8, tag="dl8")
                        nc.sync.dma_start(
                            out=dl8[:],
                            in_=dstloc[do:do + 128 * J_sb].rearrange(
                                "(p s) -> p s", s=J_sb))
                        do += 128 * J_sb
                        dl = sbp.tile([128, J_sb], BF, tag="dl")
                        nc.vector.tensor_copy(out=dl[:], in_=dl8[:])

                        if mode == "nocompute":
                            continue
                        recv = rec[:].rearrange("p (j d) -> p j d", d=elem)
                        adc = sbp.tile([128, J_sb * H], BF, tag="adc")
                        nc.vector.tensor_copy(out=adc[:], in_=adp[:])
                        e1 = sbp.tile([128, J_sb * H], F32, tag="e1")
                        nc.vector.tensor_tensor(
                            out=e1[:].rearrange("p (j h) -> p j h", h=H),
                            in0=recv[:, :, mcols:mcols + H],
                            in1=adc[:].rearrange("p (j h) -> p j h", h=H),
                            op=OP.add)
                        lr = sbp.tile([128, J_sb * H], F32, tag="lr")
                        nc.vector.tensor_scalar_mul(out=lr[:], in0=e1[:], scalar1=NEG)
                        nc.vector.tensor_tensor(out=e1[:], in0=e1[:], in1=lr[:], op=OP.max)
                        wgt = sbp.tile([128, J_sb * H], BF, tag="wgt")
                        nc.scalar.activation(out=wgt[:], in_=e1[:], func=AF.Exp)
                        msg = sbp.tile([128, J_sb * mcols], BF, tag="msg", bufs=2)
                        nc.vector.tensor_tensor(
                            out=msg[:].rearrange("p (j h f) -> p j h f", h=H, f=mcols // H),
                            in0=recv[:, :, 0:mcols].rearrange(
                                "p j (h f) -> p j h f", f=mcols // H),
                            in1=wgt[:].rearrange("p (j h) -> p j h", h=H)[:, :, :, None]
                                .to_broadcast([128, J_sb, H, mcols // H]),
                            op=OP.mult)
                        oh = sbp.tile([128, J_sb * 128], BF, tag="oh", bufs=2)
                        nc.vector.tensor_tensor(
                            out=oh[:].rearrange("p (j f) -> p j f", f=128),
                            in0=iota_sb[:][:, None, :].to_broadcast([128, J_sb, 128]),
                            in1=dl[:][:, :, None].to_broadcast([128, J_sb, 128]),
                            op=OP.is_equal)
                        for ww in sb:
                            slots = layd["win_slots"][ww]
                            if not slots:
                                continue
                            acc = psp.tile([128, mcols], F32, tag="acc", space="PSUM")
                            for i, s in enumerate(slots):
                                nc.tensor.matmul(
                                    out=acc[:],
                                    lhsT=oh[:, s * 128:(s + 1) * 128],
                                    rhs=msg[:, s * mcols:(s + 1) * mcols],
                                    start=(i == 0), stop=(i == len(slots) - 1))
                            epilogue(ww, acc)

                # ---- layer 1 epilogue ----
                def epi1(ww, acc):
                    den = sbp.tile([128, 4], F32, tag="den")
                    nc.vector.tensor_copy(
                        out=den[:],
                        in_=acc[:].rearrange("p (h f) -> p h f", f=33)[:, :, 32])
                    nc.vector.tensor_scalar_max(out=den[:], in0=den[:], scalar1=1e-30)
                    rcp = sbp.tile([128, 4], F32, tag="rcp")
                    nc.vector.reciprocal(out=rcp[:], in_=den[:])
                    x1 = sbp.tile([128, 128], F32, tag="x1")
                    accv = acc[:].rearrange("p (h f) -> p h f", f=33)
                    for h in range(HEADS):
                        nc.vector.tensor_scalar(
                            out=x1[:, h * 32:(h + 1) * 32],
                            in0=accv[:, h, 0:32],
                            scalar1=rcp[:, h:h + 1], scalar2=None, op0=OP.mult)
                    nc.vector.tensor_tensor(out=x1[:], in0=x1[:], in1=b1rep_sb[:], op=OP.add)
                    x1b = sbp.tile([128, 128], BF, tag="x1b")
                    nc.scalar.activation(out=x1b[:], in_=x1[:], func=AF.Relu)
                    tp = psp.tile([128, 128], BF, tag="tp", space="PSUM")
                    nc.tensor.transpose(out=tp[:], in_=x1b[:], identity=identbf_sb[:])
                    x1t = sbp.tile([128, 128], BF, tag="x1t")
                    nc.vector.tensor_copy(out=x1t[:], in_=tp[:])
                    nc.sync.dma_start(
                        out=x1t_dram[:, ww * 128:(ww + 1) * 128], in_=x1t[:])

                if mode not in ("noedge", "noag", "min"):
                    edge_phase(tab1_full, adtab1, D1, HEADS, 132, epi1, (0, 0))

                # ---- node phase 2 ----
                for t in range(WINDOWS if mode != "min" else 0):
                    sl = slice(t * 128, (t + 1) * 128)
                    lh2 = sbp.tile([128, 128], BF, tag="lh")
                    nc.sync.dma_start(out=lh2[:], in_=x1t_dram[:, sl])
                    acc = psp.tile([128, 35], F32, tag="acc", space="PSUM")
                    nc.tensor.matmul(out=acc[:], lhsT=lh2[:], rhs=w2a_sb[:],
                                     start=True, stop=True)
                    rec2 = sbp.tile([128, D2], BF, tag="nrec")
                    nc.vector.tensor_copy(out=rec2[:, 0:34], in_=acc[:, 0:34])
                    nc.vector.tensor_copy(out=rec2[:, 32:33], in_=ones4_sb[:, 0:1])
                    nc.sync.dma_start(out=tab2_shard[sl, :], in_=rec2[:])
                    ad1c = sbp.tile([128, 1], BF, tag="ad4")
                    nc.vector.tensor_copy(out=ad1c[:], in_=acc[:, 34:35])
                    nc.sync.dma_start(out=adtab2[sl, 0:1], in_=ad1c[:])

                if mode not in ("noag", "min"):
                    nc.gpsimd.collective_compute(
                        "AllGather", mybir.AluOpType.bypass,
                        ins=[tab2_shard[:].opt()], outs=[tab2_full[:].opt()],
                        replica_groups=[list(range(CORES))])

                # ---- layer 2 epilogue (+ fused FC head) ----
                def epi2(ww, acc):
                    den = sbp.tile([128, 1], F32, tag="den")
                    nc.vector.tensor_copy(out=den[:], in_=acc[:, 32:33])
                    nc.vector.tensor_scalar_max(out=den[:], in0=den[:], scalar1=1e-30)
                    rcp = sbp.tile([128, 1], F32, tag="rcp")
                    nc.vector.reciprocal(out=rcp[:], in_=den[:])
                    x2 = sbp.tile([128, 32], F32, tag="x2")
                    nc.vector.tensor_scalar(
                        out=x2[:], in0=acc[:, 0:32],
                        scalar1=rcp[:, 0:1], scalar2=None, op0=OP.mult)
                    nc.vector.tensor_tensor(out=x2[:], in0=x2[:], in1=b2rep_sb[:], op=OP.add)
                    x2f = sbp.tile([128, 32], F32, tag="x2f")
                    nc.scalar.activation(out=x2f[:], in_=x2[:], func=AF.Relu)
                    tp2 = psp.tile([32, 128], F32, tag="tp", space="PSUM")
                    nc.tensor.transpose(out=tp2[:], in_=x2f[:], identity=identf_sb[:])
                    # zt rows: [post 0:64 | x2T 64:96] (compute-engine APs must
                    # stay in an aligned partition subtree); fc1w rows match
                    zt = sbp.tile([96, 128], BF, tag="zt")
                    nc.vector.tensor_copy(out=zt[64:96, :], in_=tp2[:])
                    if UT_I8:
                        pt8 = sbp.tile([64, 128], I8, tag="pt8")
                        nc.sync.dma_start(out=pt8[:],
                                          in_=postt[:, ww * 128:(ww + 1) * 128])
                        nc.vector.tensor_copy(out=zt[0:64, :], in_=pt8[:])
                    else:
                        nc.sync.dma_start(out=zt[0:64, :],
                                          in_=postt[:, ww * 128:(ww + 1) * 128])
                    pa = psp.tile([32, 128], F32, tag="fc", space="PSUM")
                    nc.tensor.matmul(out=pa[:], lhsT=fc1w_sb[:], rhs=zt[:],
                                     start=True, stop=True)
                    y1 = sbp.tile([32, 128], BF, tag="y1")
                    nc.scalar.activation(out=y1[:], in_=pa[:], func=AF.Relu,
                                         bias=fc1b_sb[:])
                    pb = psp.tile([1, 128], F32, tag="fc", space="PSUM")
                    nc.tensor.matmul(out=pb[:], lhsT=fc2w_sb[:], rhs=y1[:],
                                     start=True, stop=True)
                    yo = sbp.tile([1, 128], F32, tag="yo")
                    nc.scalar.activation(out=yo[:], in_=pb[:], func=AF.Sigmoid,
                                         bias=fc2b_sb[:])
                    nc.sync.dma_start(out=out_ext[0:1, ww * 128:(ww + 1) * 128],
                                      in_=yo[:])

                if mode not in ("noedge", "noag", "min"):
                    edge_phase(tab2_full, adtab2, D2, 1, 33, epi2, (0, 0))
            if mode == "min":
                zo = sbp.tile([1, NPC_PAD], F32, tag="zo")
                nc.vector.memset(zo[:], 0.5)
                nc.sync.dma_start(out=out_ext[:], in_=zo[:])

    nc.compile()
    return nc


def _make_inputs(user_features, post_features, W1, a1s, a1d, b1,
                 W2, a2s, a2d, b2, fc1_w, fc1_b, fc2_w, fc2_b, per_core):
    uf = np.asarray(user_features, np.float32)
    pf = np.asarray(post_features, np.float32)
    W1 = np.asarray(W1, np.float32)
    W2 = np.asarray(W2, np.float32)
    a1s = np.asarray(a1s, np.float32)
    a1d = np.asarray(a1d, np.float32)
    a2s = np.asarray(a2s, np.float32)
    a2d = np.asarray(a2d, np.float32)

    w1a = np.zeros((128, 140), np.float32)
    for h in range(HEADS):
        w1a[:, h * 33:h * 33 + 32] = W1[:, h * 32:(h + 1) * 32]
        w1a[:, 132 + h] = W1[:, h * 32:(h + 1) * 32] @ a1s[h]
        w1a[:, 136 + h] = W1[:, h * 32:(h + 1) * 32] @ a1d[h]
    w2a = np.zeros((128, 35), np.float32)
    w2a[:, 0:32] = W2
    w2a[:, 33] = W2 @ a2s[0]
    w2a[:, 34] = W2 @ a2d[0]

    fc1_w = np.asarray(fc1_w, np.float32).copy()
    if UT_I8:
        # int8-quantize features; fold the dequant scales into the weights
        # (b1/fc biases are applied after aggregation, so this is exact)
        s_u = float(np.abs(uf).max()) / 127.0
        s_p = float(np.abs(pf).max()) / 127.0
        uf = np.clip(np.round(uf / s_u), -127, 127)
        pf = np.clip(np.round(pf / s_p), -127, 127)
        w1a *= s_u
        fc1_w[32:96, :] *= s_p
    # zt rows are [post | x2T], so reorder fc1w rows to match
    fc1_w = np.concatenate([fc1_w[32:96], fc1_w[0:32]], axis=0)
    FEAT_NP = np.int8 if UT_I8 else BF16
    base = dict(
        w1a=w1a.astype(BF16), w2a=w2a.astype(BF16),
        fc1w=fc1_w.astype(BF16),
        fc2w=np.asarray(fc2_w, np.float32).astype(BF16),
        fc1b=np.asarray(fc1_b, np.float32).reshape(32, 1).copy(),
        fc2b=np.asarray(fc2_b, np.float32).reshape(1, 1).copy(),
        b1row=np.asarray(b1, np.float32).reshape(1, 128).copy(),
        b2row=np.asarray(b2, np.float32).reshape(1, 32).copy(),
    )
    in_maps = []
    for c in range(CORES):
        sl = slice(c * NPC, (c + 1) * NPC)
        ut = np.zeros((128, NPC_PAD), np.float32)
        ut[:, :NPC] = uf[sl].T
        postt = np.zeros((F_POST, NPC_PAD), np.float32)
        postt[:, :NPC] = pf[sl].T
        m = dict(base)
        m["ut"] = ut.astype(FEAT_NP)
        m["postt"] = postt.astype(FEAT_NP)
        m.update(per_core[c])
        in_maps.append(m)
    return in_maps


_CACHE = {}
LAST_EXEC_NS = None


def kernel(**inputs):
    from concourse.bass_utils import run_bass_kernel_spmd
    ei = np.asarray(inputs["edge_index"])
    static, per_core = preprocess(ei)
    blob_sizes = dict(ad=len(per_core[0]["dstloct"]),
                      dl=len(per_core[0]["dstloc"]))
    in_maps = _make_inputs(
        inputs["user_features"], inputs["post_features"],
        inputs["W1"], inputs["a1s"], inputs["a1d"], inputs["b1"],
        inputs["W2"], inputs["a2s"], inputs["a2d"], inputs["b2"],
        inputs["fc1_w"], inputs["fc1_b"], inputs["fc2_w"], inputs["fc2_b"],
        per_core)
    key = (static["tot_cols"], blob_sizes["ad"], blob_sizes["dl"])
    if key not in _CACHE:
        _CACHE[key] = build_program(static, blob_sizes)
    nc = _CACHE[key]
    import os
    trace = bool(os.environ.get("BASS_KERNEL_TRACE"))
    r = run_bass_kernel_spmd(nc, in_maps, list(range(CORES)), trace=trace)
    global LAST_EXEC_NS
    LAST_EXEC_NS = r.exec_time_ns
    out = np.empty((N, 1), np.float32)
    for c in range(CORES):
        out[c * NPC:(c + 1) * NPC, 0] = r.results[c]["out"][0, :NPC]
    return out


# revision 11
# speedup vs baseline: 2.0301x; 1.0326x over previous
"""GAT model (2-layer GAT + FC head) on 8 Trainium2 NeuronCores.

Strategy: destination-sharded. Each core owns 12544 (padded) dst nodes
= 98 windows of 128. Edges live on their dst's core, sorted into
(window, src-chunk) groups. Node phase computes per-node tables
[h | as] (bf16) sharded + AllGather; ad values stay core-local.
Edge phase: dma_gather of 512B records by src (int16 idx over 4
chunks of 25088 rows) + broadcast of dst-local ids; per-edge
softmax weights w = exp(leakyrelu(as+ad)) (no segment-max needed:
scores are bounded, exp cannot overflow in f32); messages
msg = w * [h | 1] scattered into per-window PSUM via one-hot matmuls
(one-hot built in bulk on DVE from iota==dstloc). Denominator rides
the matmul via the record's ones-column. FC head fused per window.

Host->device upload is the wall-clock bottleneck (axon PJRT tunnel),
so inputs are minimized: gather indices uploaded unreplicated
[16, cols] and tiled 8x across partitions on device into one
resident SBUF tile; dst-local ids as uint8 (cast on device);
node/post features as scaled int8 (cast to bf16 on device,
scales folded into w1a/fc1w on host); iota /
identity / replicated-bias constants built on device.
"""
import sys
import numpy as np
import ml_dtypes

sys.path.insert(0, "/opt/trn_rl_repo")

BF16 = ml_dtypes.bfloat16

N = 100000
E_RAW = 1600000
F_USER = 128
F_POST = 64
HID = 32
HEADS = 4
NEG = 0.2
CORES = 8
NPC = 12500                 # real nodes per core
NPC_PAD = 12544             # 98 * 128
WINDOWS = 98
N_PAD = NPC_PAD * CORES     # 100352
NCHUNK = 4
CHUNK = N_PAD // NCHUNK     # 25088
SW = 2                      # windows per superblock
D1 = 256                    # table1 row elems (bf16): [hblk 132 | as 4 | pad]
D2 = 128                    # table2 row elems: [h2blk 33 | as2 1 | pad]
DAD = 128                   # ad table row elems: [ad .. | pad]
UT_I8 = True                # upload node/post features as scaled int8


def _layout(tot_cols, adlen, dllen):
    """Byte layout of the single packed input blob (64B-aligned fields)."""
    fb = 1 if UT_I8 else 2
    fields = [
        ("srcidx", 16 * tot_cols * 2),
        ("ut", 128 * NPC_PAD * fb),
        ("postt", F_POST * NPC_PAD * fb),
        ("dstloct", adlen),
        ("dstloc", dllen),
        ("w1a", 128 * 140 * 2),
        ("w2a", 128 * 35 * 2),
        ("fc1w", 96 * 32 * 2),
        ("fc2w", 32 * 1 * 2),
        ("fc1b", 32 * 4),
        ("fc2b", 4),
        ("b1row", 128 * 4),
        ("b2row", 32 * 4),
    ]
    lay = {}
    off = 0
    for name, nb in fields:
        lay[name] = (off, nb)
        off += (nb + 63) // 64 * 64
    return lay, off


def _g(v):
    """original node id -> padded global id"""
    return (v // NPC) * NPC_PAD + (v % NPC)


def preprocess(edge_index):
    """Returns (static, per_core) where static describes the shared program
    shape and per_core[c] holds the input blobs."""
    src = np.asarray(edge_index[0], dtype=np.int64)
    dst = np.asarray(edge_index[1], dtype=np.int64)
    loops = np.arange(N, dtype=np.int64)
    src = np.concatenate([src, loops])
    dst = np.concatenate([dst, loops])
    sp = _g(src)
    core = dst // NPC
    dloc_c = dst % NPC                      # 0..12499
    w = dloc_c // 128
    dloc_w = dloc_c % 128
    ch = sp // CHUNK
    srel = sp % CHUNK

    key = ((core * WINDOWS + w) * NCHUNK + ch).astype(np.int64)
    counts = np.bincount(key, minlength=CORES * WINDOWS * NCHUNK)
    counts = counts.reshape(CORES, WINDOWS, NCHUNK)
    maxc = counts.max(axis=0)               # [WINDOWS, NCHUNK]
    J = -(-maxc // 128)                     # ceil div; may be 0

    # superblocks
    sbs = [list(range(s, min(s + SW, WINDOWS))) for s in range(0, WINDOWS, SW)]

    # static slot layout per sb: chunk-major, then window
    sb_layout = []   # per sb: dict(ch -> [(w, slot_off_in_sb, J_w_ch)]), J_sb, per-window slot list
    for sb in sbs:
        off = 0
        per_ch = []
        win_slots = {ww: [] for ww in sb}
        for c in range(NCHUNK):
            groups = []
            for ww in sb:
                j = int(J[ww, c])
                if j == 0:
                    continue
                groups.append((ww, off, j))
                win_slots[ww].extend(range(off, off + j))
                off += j
            per_ch.append(groups)
        sb_layout.append(dict(per_ch=per_ch, J_sb=off, win_slots=win_slots))

    # static column offsets of each (sb, chunk) block in the resident idx tile
    idx_colo = []
    tot_cols = 0
    for si, sb in enumerate(sbs):
        cc = []
        for cidx in range(NCHUNK):
            groups = sb_layout[si]["per_ch"][cidx]
            Jch = sum(j for (_, _, j) in groups)
            cc.append(tot_cols)
            tot_cols += 8 * Jch             # (128*Jch)/16 columns
        idx_colo.append(cc)

    order = np.lexsort((srel, ch, w, core))
    so, wo, cho, srelo, dlwo, dlco = (
        x[order] for x in (sp, w, ch, srel, dloc_w, dloc_c))
    coreo = core[order]
    # group start offsets in sorted array per (core, w, ch)
    keyo = ((coreo * WINDOWS + wo) * NCHUNK + cho)
    starts = np.searchsorted(keyo, np.arange(CORES * WINDOWS * NCHUNK))
    ends = np.searchsorted(keyo, np.arange(CORES * WINDOWS * NCHUNK) + 1)

    per_core = []
    for c in range(CORES):
        idx_cols = []      # [16, cols] blocks, horizontally concatenated
        ad_blob = []
        dl_blob = []
        for si, sb in enumerate(sbs):
            lay = sb_layout[si]
            J_sb = lay["J_sb"]
            dl_arr = np.full((128, J_sb), 255, np.uint8)
            for cidx in range(NCHUNK):
                groups = lay["per_ch"][cidx]
                if not groups:
                    continue
                G = 128 * sum(j for (_, _, j) in groups)
                idx_flat = np.zeros(G, np.int16)
                off0 = groups[0][1]
                for (ww, soff, j) in groups:
                    gi = (c * WINDOWS + ww) * NCHUNK + cidx
                    s0, s1 = int(starts[gi]), int(ends[gi])
                    n = s1 - s0
                    gbase = (soff - off0) * 128
                    idx_flat[gbase:gbase + n] = srelo[s0:s1].astype(np.int16)
                    k = np.arange(n)
                    dl_arr[k % 128, soff + k // 128] = dlwo[s0:s1]
                idx_cols.append(idx_flat.reshape(G // 16, 16).T)  # [16, G/16]
            ad_blob.append(dl_arr.T.ravel())   # dstlocT flat [J_sb*128] u8
            dl_blob.append(dl_arr.ravel())
        per_core.append(dict(
            srcidx=np.ascontiguousarray(np.concatenate(idx_cols, axis=1)),
            dstloct=np.concatenate(ad_blob),
            dstloc=np.concatenate(dl_blob),
        ))
    static = dict(J=J, sbs=sbs, sb_layout=sb_layout, idx_colo=idx_colo,
                  tot_cols=tot_cols)
    return static, per_core


def build_program(static, blob_sizes):
    import os
    mode = os.environ.get("KMODE", "full")
    import concourse.bass as bass
    import concourse.bacc as bacc
    import concourse.tile as tile
    from concourse import mybir

    F32, BF, I16 = mybir.dt.float32, mybir.dt.bfloat16, mybir.dt.int16
    U8, I32, I8 = mybir.dt.uint8, mybir.dt.int32, mybir.dt.int8
    FEAT = I8 if UT_I8 else BF
    AF = mybir.ActivationFunctionType
    OP = mybir.AluOpType
    sbs, lay = static["sbs"], static["sb_layout"]
    idx_colo, TOT_COLS = static["idx_colo"], static["tot_cols"]

    reps = int(os.environ.get("KREPS", "1"))
    nc = bacc.Bacc("TRN2", target_bir_lowering=False, debug=False)
    P = nc.declare_dram_parameter
    LAYT, BLOBN = _layout(TOT_COLS, blob_sizes["ad"], blob_sizes["dl"])
    blob = P("blob", [BLOBN], U8, isOutput=False)
    out_ext = P("out", [1, NPC_PAD], F32, isOutput=True)

    def fld(name, dt=None, cols=None):
        off, nb = LAYT[name]
        ap = blob[off:off + nb]
        if dt is not None:
            ap = ap.bitcast(dt)
        if cols is not None:
            ap = ap.rearrange("(p s) -> p s", s=cols)
        return ap

    ut = fld("ut", FEAT, NPC_PAD)
    postt = fld("postt", FEAT, NPC_PAD)
    srcidx = fld("srcidx", I16, TOT_COLS)
    dstloct = fld("dstloct")
    dstloc = fld("dstloc")

    with tile.TileContext(nc) as tc:
        with (
            tc.tile_pool(name="cst", bufs=1) as cst,
            tc.tile_pool(name="sb", bufs=3) as sbp,
            tc.tile_pool(name="ps", bufs=2, space="PSUM") as psp,
            tc.tile_pool(name="dr", bufs=1, space="DRAM") as dr,
        ):
            tab1_shard = dr.tile([NPC_PAD, D1], BF)
            adtab1 = dr.tile([NPC_PAD, DAD], BF)
            tab2_shard = dr.tile([NPC_PAD, D2], BF)
            adtab2 = dr.tile([NPC_PAD, DAD], BF)
            x1t_dram = dr.tile([128, NPC_PAD], BF)

            w1a_sb = cst.tile([128, 140], BF)
            w2a_sb = cst.tile([128, 35], BF)
            fc1w_sb = cst.tile([96, 32], BF)
            fc2w_sb = cst.tile([32, 1], BF)
            fc1b_sb = cst.tile([32, 1], F32)
            fc2b_sb = cst.tile([1, 1], F32)
            b1rep_sb = cst.tile([128, 128], F32)
            b2rep_sb = cst.tile([128, 32], F32)
            for t, name, cols in [(w1a_sb, "w1a", 140), (w2a_sb, "w2a", 35),
                                  (fc1w_sb, "fc1w", 32), (fc2w_sb, "fc2w", 1),
                                  (fc1b_sb, "fc1b", 1), (fc2b_sb, "fc2b", 1)]:
                dt = F32 if name in ("fc1b", "fc2b") else BF
                nc.sync.dma_start(out=t[:], in_=fld(name, dt, cols))
            nc.sync.dma_start(
                out=b1rep_sb[:],
                in_=fld("b1row", F32)[None, :].to_broadcast([128, 128]))
            nc.sync.dma_start(
                out=b2rep_sb[:],
                in_=fld("b2row", F32)[None, :].to_broadcast([128, 32]))

            # on-device constants: iota row/col, identities, ones
            iotar_i = cst.tile([128, 128], I16)
            nc.gpsimd.iota(iotar_i[:], pattern=[[1, 128]], channel_multiplier=0)
            iota_sb = cst.tile([128, 128], BF)
            nc.vector.tensor_copy(out=iota_sb[:], in_=iotar_i[:])
            iotac_i = cst.tile([128, 1], I32)
            nc.gpsimd.iota(iotac_i[:], pattern=[[0, 1]], channel_multiplier=1)
            iotacol_sb = cst.tile([128, 1], F32)
            nc.vector.tensor_copy(out=iotacol_sb[:], in_=iotac_i[:])
            identbf_sb = cst.tile([128, 128], BF)
            nc.vector.tensor_scalar(
                out=identbf_sb[:], in0=iota_sb[:], scalar1=iotacol_sb[:, 0:1],
                scalar2=None, op0=OP.is_equal)
            identf_sb = cst.tile([128, 128], F32)
            nc.vector.tensor_scalar(
                out=identf_sb[:], in0=iota_sb[:], scalar1=iotacol_sb[:, 0:1],
                scalar2=None, op0=OP.is_equal)
            ones4_sb = cst.tile([128, 4], BF)
            nc.vector.memset(ones4_sb[:], 1.0)

            for _rep in range(reps):
                tab1_full = dr.tile([N_PAD, D1], BF, addr_space="Shared",
                                    name=f"tab1_full_r{_rep}")
                tab2_full = dr.tile([N_PAD, D2], BF, addr_space="Shared",
                                    name=f"tab2_full_r{_rep}")
                # ---- node phase 1: tables for layer 1 ----
                for t in range(WINDOWS if mode != "min" else 0):
                    sl = slice(t * 128, (t + 1) * 128)
                    if UT_I8:
                        lh8 = sbp.tile([128, 128], I8, tag="lh8")
                        nc.sync.dma_start(out=lh8[:], in_=ut[:, sl])
                        lh = sbp.tile([128, 128], BF, tag="lh")
                        nc.vector.tensor_copy(out=lh[:], in_=lh8[:])
                    else:
                        lh = sbp.tile([128, 128], BF, tag="lh")
                        nc.sync.dma_start(out=lh[:], in_=ut[:, sl])
                    acc = psp.tile([128, 140], F32, tag="acc", space="PSUM")
                    nc.tensor.matmul(out=acc[:], lhsT=lh[:], rhs=w1a_sb[:],
                                     start=True, stop=True)
                    rec = sbp.tile([128, D1], BF, tag="nrec")
                    nc.vector.tensor_copy(out=rec[:, 0:136], in_=acc[:, 0:136])
                    nc.vector.tensor_copy(
                        out=rec[:, 0:132].rearrange("p (h f) -> p h f", f=33)[:, :, 32],
                        in_=ones4_sb[:])
                    nc.sync.dma_start(out=tab1_shard[sl, :], in_=rec[:])
                    ad4 = sbp.tile([128, 4], BF, tag="ad4")
                    nc.vector.tensor_copy(out=ad4[:], in_=acc[:, 136:140])
                    nc.sync.dma_start(out=adtab1[sl, 0:4], in_=ad4[:])

                if mode not in ("noag", "min"):
                    nc.gpsimd.collective_compute(
                        "AllGather", mybir.AluOpType.bypass,
                        ins=[tab1_shard[:].opt()], outs=[tab1_full[:].opt()],
                        replica_groups=[list(range(CORES))])

                # ---- generic edge phase ----
                def edge_phase(tabfull, adtab, elem, H, mcols, epilogue, blob_offs):
                    ao, do = blob_offs
                    for si, sb in enumerate(sbs):
                        layd = lay[si]
                        J_sb = layd["J_sb"]
                        c0 = idx_colo[si][0]
                        c1 = (idx_colo[si + 1][0] if si + 1 < len(sbs)
                              else TOT_COLS)
                        sbcols = c1 - c0
                        idxt = sbp.tile([128, sbcols], I16, tag="idxt", bufs=2)
                        for r in range(8):
                            nc.sync.dma_start(
                                out=idxt[16 * r:16 * r + 16, :],
                                in_=srcidx[:, c0:c1])
                        rec = sbp.tile([128, J_sb * elem], BF, tag="erec", bufs=2)
                        for cidx in range(NCHUNK):
                            groups = layd["per_ch"][cidx]
                            if not groups:
                                continue
                            Jch = sum(j for (_, _, j) in groups)
                            off0 = groups[0][1]
                            G = 128 * Jch
                            if mode in ("nogather",):
                                continue
                            nc.gpsimd.dma_gather(
                                out_ap=rec[:, off0 * elem:(off0 + Jch) * elem]
                                    .rearrange("p (j d) -> p j d", d=elem),
                                in_ap=tabfull[cidx * CHUNK:(cidx + 1) * CHUNK, :],
                                idxs_ap=idxt[:, idx_colo[si][cidx] - c0:
                                             idx_colo[si][cidx] - c0 + 8 * Jch],
                                num_idxs=G, num_idxs_reg=G,
                                elem_size=elem, single_packet=False)
                        Gad = J_sb * 128
                        dtr8 = sbp.tile([128, Gad], U8, tag="adE8", bufs=2)
                        nc.sync.dma_start(
                            out=dtr8[:],
                            in_=dstloct[ao:ao + Gad][None, :].to_broadcast([128, Gad]))
                        ao += Gad
                        dtr = sbp.tile([128, Gad], BF, tag="adE", bufs=2)
                        nc.vector.tensor_copy(out=dtr[:], in_=dtr8[:])
                        ohT = sbp.tile([128, Gad], BF, tag="ohT", bufs=2)
                        nc.vector.tensor_scalar(
                            out=ohT[:], in0=dtr[:], scalar1=iotacol_sb[:, 0:1],
                            scalar2=None, op0=OP.is_equal)
                        adp = psp.tile([128, J_sb * H], F32, tag="adp", space="PSUM")
                        for ww2 in sb:
                            adw = sbp.tile([128, H], BF, tag="adw")
                            nc.sync.dma_start(
                                out=adw[:], in_=adtab[ww2 * 128:(ww2 + 1) * 128, 0:H])
                            for s_ in layd["win_slots"][ww2]:
                                nc.tensor.matmul(
                                    out=adp[:, s_ * H:(s_ + 1) * H],
                                    lhsT=ohT[:, s_ * 128:(s_ + 1) * 128],
                                    rhs=adw[:], start=True, stop=True)
                        dl8 = sbp.tile([128, J_sb], U8, tag="dl8")
                        nc.sync.dma_start(
                            out=dl8[:],
                            in_=dstloc[do:do + 128 * J_sb].rearrange(
                                "(p s) -> p s", s=J_sb))
                        do += 128 * J_sb
                        dl = sbp.tile([128, J_sb], BF, tag="dl")
                        nc.vector.tensor_copy(out=dl[:], in_=dl8[:])

                        if mode == "nocompute":
                            continue
                        recv = rec[:].rearrange("p (j d) -> p j d", d=elem)
                        adc = sbp.tile([128, J_sb * H], BF, tag="adc")
                        nc.vector.tensor_copy(out=adc[:], in_=adp[:])
                        e1 = sbp.tile([128, J_sb * H], F32, tag="e1")
                        nc.vector.tensor_tensor(
                            out=e1[:].rearrange("p (j h) -> p j h", h=H),
                            in0=recv[:, :, mcols:mcols + H],
                            in1=adc[:].rearrange("p (j h) -> p j h", h=H),
                            op=OP.add)
                        lr = sbp.tile([128, J_sb * H], F32, tag="lr")
                        nc.vector.tensor_scalar_mul(out=lr[:], in0=e1[:], scalar1=NEG)
                        nc.vector.tensor_tensor(out=e1[:], in0=e1[:], in1=lr[:], op=OP.max)
                        wgt = sbp.tile([128, J_sb * H], BF, tag="wgt")
                        nc.scalar.activation(out=wgt[:], in_=e1[:], func=AF.Exp)
                        msg = sbp.tile([128, J_sb * mcols], BF, tag="msg", bufs=2)
                        nc.vector.tensor_tensor(
                            out=msg[:].rearrange("p (j h f) -> p j h f", h=H, f=mcols // H),
                            in0=recv[:, :, 0:mcols].rearrange(
                                "p j (h f) -> p j h f", f=mcols // H),
                            in1=wgt[:].rearrange("p (j h) -> p j h", h=H)[:, :, :, None]
                                .to_broadcast([128, J_sb, H, mcols // H]),
                            op=OP.mult)
                        oh = sbp.tile([128, J_sb * 128], BF, tag="oh", bufs=2)
                        nc.vector.tensor_tensor(
                            out=oh[:].rearrange("p (j f) -> p j f", f=128),
                            in0=iota_sb[:][:, None, :].to_broadcast([128, J_sb, 128]),
                            in1=dl[:][:, :, None].to_broadcast([128, J_sb, 128]),
                            op=OP.is_equal)
                        for ww in sb:
                            slots = layd["win_slots"][ww]
                            if not slots:
                                continue
                            acc = psp.tile([128, mcols], F32, tag="acc", space="PSUM")
                            for i, s in enumerate(slots):
                                nc.tensor.matmul(
                                    out=acc[:],
                                    lhsT=oh[:, s * 128:(s + 1) * 128],
                                    rhs=msg[:, s * mcols:(s + 1) * mcols],
                                    start=(i == 0), stop=(i == len(slots) - 1))
                            epilogue(ww, acc)

                # ---- layer 1 epilogue ----
                def epi1(ww, acc):
                    den = sbp.tile([128, 4], F32, tag="den")
                    nc.vector.tensor_copy(
                        out=den[:],
                        in_=acc[:].rearrange("p (h f) -> p h f", f=33)[:, :, 32])
                    nc.vector.tensor_scalar_max(out=den[:], in0=den[:], scalar1=1e-30)
                    rcp = sbp.tile([128, 4], F32, tag="rcp")
                    nc.vector.reciprocal(out=rcp[:], in_=den[:])
                    x1 = sbp.tile([128, 128], F32, tag="x1")
                    accv = acc[:].rearrange("p (h f) -> p h f", f=33)
                    for h in range(HEADS):
                        nc.vector.tensor_scalar(
                            out=x1[:, h * 32:(h + 1) * 32],
                            in0=accv[:, h, 0:32],
                            scalar1=rcp[:, h:h + 1], scalar2=None, op0=OP.mult)
                    nc.vector.tensor_tensor(out=x1[:], in0=x1[:], in1=b1rep_sb[:], op=OP.add)
                    x1b = sbp.tile([128, 128], BF, tag="x1b")
                    nc.scalar.activation(out=x1b[:], in_=x1[:], func=AF.Relu)
                    tp = psp.tile([128, 128], BF, tag="tp", space="PSUM")
                    nc.tensor.transpose(out=tp[:], in_=x1b[:], identity=identbf_sb[:])
                    x1t = sbp.tile([128, 128], BF, tag="x1t")
                    nc.vector.tensor_copy(out=x1t[:], in_=tp[:])
                    nc.sync.dma_start(
                        out=x1t_dram[:, ww * 128:(ww + 1) * 128], in_=x1t[:])

                if mode not in ("noedge", "noag", "min"):
                    edge_phase(tab1_full, adtab1, D1, HEADS, 132, epi1, (0, 0))

                # ---- node phase 2 ----
                for t in range(WINDOWS if mode != "min" else 0):
                    sl = slice(t * 128, (t + 1) * 128)
                    lh2 = sbp.tile([128, 128], BF, tag="lh")
                    nc.sync.dma_start(out=lh2[:], in_=x1t_dram[:, sl])
                    acc = psp.tile([128, 35], F32, tag="acc", space="PSUM")
                    nc.tensor.matmul(out=acc[:], lhsT=lh2[:], rhs=w2a_sb[:],
                                     start=True, stop=True)
                    rec2 = sbp.tile([128, D2], BF, tag="nrec")
                    nc.vector.tensor_copy(out=rec2[:, 0:34], in_=acc[:, 0:34])
                    nc.vector.tensor_copy(out=rec2[:, 32:33], in_=ones4_sb[:, 0:1])
                    nc.sync.dma_start(out=tab2_shard[sl, :], in_=rec2[:])
                    ad1c = sbp.tile([128, 1], BF, tag="ad4")
                    nc.vector.tensor_copy(out=ad1c[:], in_=acc[:, 34:35])
                    nc.sync.dma_start(out=adtab2[sl, 0:1], in_=ad1c[:])

                if mode not in ("noag", "min"):
                    nc.gpsimd.collective_compute(
                        "AllGather", mybir.AluOpType.bypass,
                        ins=[tab2_shard[:].opt()], outs=[tab2_full[:].opt()],
                        replica_groups=[list(range(CORES))])

                # ---- layer 2 epilogue (+ fused FC head) ----
                def epi2(ww, acc):
                    den = sbp.tile([128, 1], F32, tag="den")
                    nc.vector.tensor_copy(out=den[:], in_=acc[:, 32:33])
                    nc.vector.tensor_scalar_max(out=den[:], in0=den[:], scalar1=1e-30)
                    rcp = sbp.tile([128, 1], F32, tag="rcp")
                    nc.vector.reciprocal(out=rcp[:], in_=den[:])
                    x2 = sbp.tile([128, 32], F32, tag="x2")
                    nc.vector.tensor_scalar(
                        out=x2[:], in0=acc[:, 0:32],
                        scalar1=rcp[:, 0:1], scalar2=None, op0=OP.mult)
                    nc.vector.tensor_tensor(out=x2[:], in0=x2[:], in1=b2rep_sb[:], op=OP.add)
                    x2f = sbp.tile([128, 32], F32, tag="x2f")
                    nc.scalar.activation(out=x2f[:], in_=x2[:], func=AF.Relu)
                    tp2 = psp.tile([32, 128], F32, tag="tp", space="PSUM")
                    nc.tensor.transpose(out=tp2[:], in_=x2f[:], identity=identf_sb[:])
                    # zt rows: [post 0:64 | x2T 64:96] (compute-engine APs must
                    # stay in an aligned partition subtree); fc1w rows match
                    zt = sbp.tile([96, 128], BF, tag="zt")
                    nc.vector.tensor_copy(out=zt[64:96, :], in_=tp2[:])
                    if UT_I8:
                        pt8 = sbp.tile([64, 128], I8, tag="pt8")
                        nc.sync.dma_start(out=pt8[:],
                                          in_=postt[:, ww * 128:(ww + 1) * 128])
                        nc.vector.tensor_copy(out=zt[0:64, :], in_=pt8[:])
                    else:
                        nc.sync.dma_start(out=zt[0:64, :],
                                          in_=postt[:, ww * 128:(ww + 1) * 128])
                    pa = psp.tile([32, 128], F32, tag="fc", space="PSUM")
                    nc.tensor.matmul(out=pa[:], lhsT=fc1w_sb[:], rhs=zt[:],
                                     start=True, stop=True)
                    y1 = sbp.tile([32, 128], BF, tag="y1")
                    nc.scalar.activation(out=y1[:], in_=pa[:], func=AF.Relu,
                                         bias=fc1b_sb[:])
                    pb = psp.tile([1, 128], F32, tag="fc", space="PSUM")
                    nc.tensor.matmul(out=pb[:], lhsT=fc2w_sb[:], rhs=y1[:],
                                     start=True, stop=True)
                    yo = sbp.tile([1, 128], F32, tag="yo")
                    nc.scalar.activation(out=yo[:], in_=pb[:], func=AF.Sigmoid,
                                         bias=fc2b_sb[:])
                    nc.sync.dma_start(out=out_ext[0:1, ww * 128:(ww + 1) * 128],
                                      in_=yo[:])

                if mode not in ("noedge", "noag", "min"):
                    edge_phase(tab2_full, adtab2, D2, 1, 33, epi2, (0, 0))
            if mode == "min":
                zo = sbp.tile([1, NPC_PAD], F32, tag="zo")
                nc.vector.memset(zo[:], 0.5)
                nc.sync.dma_start(out=out_ext[:], in_=zo[:])

    nc.compile()
    return nc


def _make_inputs(user_features, post_features, W1, a1s, a1d, b1,
                 W2, a2s, a2d, b2, fc1_w, fc1_b, fc2_w, fc2_b, per_core):
    uf = np.asarray(user_features, np.float32)
    pf = np.asarray(post_features, np.float32)
    W1 = np.asarray(W1, np.float32)
    W2 = np.asarray(W2, np.float32)
    a1s = np.asarray(a1s, np.float32)
    a1d = np.asarray(a1d, np.float32)
    a2s = np.asarray(a2s, np.float32)
    a2d = np.asarray(a2d, np.float32)

    w1a = np.zeros((128, 140), np.float32)
    for h in range(HEADS):
        w1a[:, h * 33:h * 33 + 32] = W1[:, h * 32:(h + 1) * 32]
        w1a[:, 132 + h] = W1[:, h * 32:(h + 1) * 32] @ a1s[h]
        w1a[:, 136 + h] = W1[:, h * 32:(h + 1) * 32] @ a1d[h]
    w2a = np.zeros((128, 35), np.float32)
    w2a[:, 0:32] = W2
    w2a[:, 33] = W2 @ a2s[0]
    w2a[:, 34] = W2 @ a2d[0]

    fc1_w = np.asarray(fc1_w, np.float32).copy()
    if UT_I8:
        # int8-quantize features; fold the dequant scales into the weights
        # (b1/fc biases are applied after aggregation, so this is exact)
        s_u = float(np.abs(uf).max()) / 127.0
        s_p = float(np.abs(pf).max()) / 127.0
        uf = np.clip(np.round(uf / s_u), -127, 127)
        pf = np.clip(np.round(pf / s_p), -127, 127)
        w1a *= s_u
        fc1_w[32:96, :] *= s_p
    # zt rows are [post | x2T], so reorder fc1w rows to match
    fc1_w = np.concatenate([fc1_w[32:96], fc1_w[0:32]], axis=0)
    FEAT_NP = np.int8 if UT_I8 else BF16

    lay, BLOBN = _layout(per_core[0]["srcidx"].shape[1],
                         len(per_core[0]["dstloct"]),
                         len(per_core[0]["dstloc"]))

    def pack(blob, name, arr):
        off, nb = lay[name]
        raw = np.ascontiguousarray(arr).view(np.uint8).ravel()
        assert raw.nbytes == nb, (name, raw.nbytes, nb)
        blob[off:off + nb] = raw

    consts = [
        ("w1a", w1a.astype(BF16)),
        ("w2a", w2a.astype(BF16)),
        ("fc1w", fc1_w.astype(BF16)),
        ("fc2w", np.asarray(fc2_w, np.float32).astype(BF16)),
        ("fc1b", np.asarray(fc1_b, np.float32).reshape(32, 1).copy()),
        ("fc2b", np.asarray(fc2_b, np.float32).reshape(1, 1).copy()),
        ("b1row", np.asarray(b1, np.float32).reshape(1, 128).copy()),
        ("b2row", np.asarray(b2, np.float32).reshape(1, 32).copy()),
    ]
    in_maps = []
    for c in range(CORES):
        sl = slice(c * NPC, (c + 1) * NPC)
        ut = np.zeros((128, NPC_PAD), np.float32)
        ut[:, :NPC] = uf[sl].T
        postt = np.zeros((F_POST, NPC_PAD), np.float32)
        postt[:, :NPC] = pf[sl].T
        blob = np.zeros(BLOBN, np.uint8)
        pack(blob, "ut", ut.astype(FEAT_NP))
        pack(blob, "postt", postt.astype(FEAT_NP))
        for name, arr in consts:
            pack(blob, name, arr)
        for name in ("srcidx", "dstloct", "dstloc"):
            pack(blob, name, per_core[c][name])
        in_maps.append(dict(blob=blob))
    return in_maps


_CACHE = {}
LAST_EXEC_NS = None


def kernel(**inputs):
    from concourse.bass_utils import run_bass_kernel_spmd
    ei = np.asarray(inputs["edge_index"])
    static, per_core = preprocess(ei)
    blob_sizes = dict(ad=len(per_core[0]["dstloct"]),
                      dl=len(per_core[0]["dstloc"]))
    in_maps = _make_inputs(
        inputs["user_features"], inputs["post_features"],
        inputs["W1"], inputs["a1s"], inputs["a1d"], inputs["b1"],
        inputs["W2"], inputs["a2s"], inputs["a2d"], inputs["b2"],
        inputs["fc1_w"], inputs["fc1_b"], inputs["fc2_w"], inputs["fc2_b"],
        per_core)
    key = (static["tot_cols"], blob_sizes["ad"], blob_sizes["dl"])
    if key not in _CACHE:
        _CACHE[key] = build_program(static, blob_sizes)
    nc = _CACHE[key]
    import os
    trace = bool(os.environ.get("BASS_KERNEL_TRACE"))
    r = run_bass_kernel_spmd(nc, in_maps, list(range(CORES)), trace=trace)
    global LAST_EXEC_NS
    LAST_EXEC_NS = r.exec_time_ns
    out = np.empty((N, 1), np.float32)
    for c in range(CORES):
        out[c * NPC:(c + 1) * NPC, 0] = r.results[c]["out"][0, :NPC]
    return out


# revision 12
# speedup vs baseline: 4.1726x; 2.0553x over previous
"""GAT model (2-layer GAT + FC head) on 8 Trainium2 NeuronCores.

Strategy: destination-sharded. Each core owns 12544 (padded) dst nodes
= 98 windows of 128. Edges live on their dst's core, sorted into
(window, src-chunk) groups. Node phase computes per-node tables
[h | as] (bf16) sharded + AllGather; ad values stay core-local.
Edge phase: dma_gather of 512B records by src (int16 idx over 4
chunks of 25088 rows) + broadcast of dst-local ids; per-edge
softmax weights w = exp(leakyrelu(as+ad)) (no segment-max needed:
scores are bounded, exp cannot overflow in f32); messages
msg = w * [h | 1] scattered into per-window PSUM via one-hot matmuls
(one-hot built in bulk on DVE from iota==dstloc). Denominator rides
the matmul via the record's ones-column. FC head fused per window.

Host->device upload is the wall-clock bottleneck (axon PJRT tunnel),
so inputs are minimized: gather indices uploaded unreplicated
[16, cols] and tiled 8x across partitions on device into one
resident SBUF tile; dst-local ids as uint8 (cast on device);
node/post features as scaled int8 (cast to bf16 on device,
scales folded into w1a/fc1w on host); iota /
identity / replicated-bias constants built on device.
"""
import sys
import numpy as np
import ml_dtypes

sys.path.insert(0, "/opt/trn_rl_repo")

# Persistent XLA compilation cache: run_bass_kernel_spmd rebuilds its jit
# closure every call, which otherwise re-compiles the (identical) wrapper
# executable each time (~1.1 s/call). With the cache, repeat calls
# deserialize instead.
try:
    import jax as _jax
    _jax.config.update("jax_compilation_cache_dir", "/tmp/jax_comp_cache")
    _jax.config.update("jax_persistent_cache_min_compile_time_secs", 0.0)
    _jax.config.update("jax_persistent_cache_min_entry_size_bytes", 0)
except Exception:
    pass

BF16 = ml_dtypes.bfloat16

N = 100000
E_RAW = 1600000
F_USER = 128
F_POST = 64
HID = 32
HEADS = 4
NEG = 0.2
CORES = 8
NPC = 12500                 # real nodes per core
NPC_PAD = 12544             # 98 * 128
WINDOWS = 98
N_PAD = NPC_PAD * CORES     # 100352
NCHUNK = 4
CHUNK = N_PAD // NCHUNK     # 25088
SW = 2                      # windows per superblock
D1 = 256                    # table1 row elems (bf16): [hblk 132 | as 4 | pad]
D2 = 128                    # table2 row elems: [h2blk 33 | as2 1 | pad]
DAD = 128                   # ad table row elems: [ad .. | pad]
UT_I8 = True                # upload node/post features as scaled int8


def _layout(tot_cols, adlen, dllen):
    """Byte layout of the single packed input blob (64B-aligned fields)."""
    fb = 1 if UT_I8 else 2
    fields = [
        ("srcidx", 16 * tot_cols * 2),
        ("ut", 128 * NPC_PAD * fb),
        ("postt", F_POST * NPC_PAD * fb),
        ("dstloct", adlen),
        ("dstloc", dllen),
        ("w1a", 128 * 140 * 2),
        ("w2a", 128 * 35 * 2),
        ("fc1w", 96 * 32 * 2),
        ("fc2w", 32 * 1 * 2),
        ("fc1b", 32 * 4),
        ("fc2b", 4),
        ("b1row", 128 * 4),
        ("b2row", 32 * 4),
    ]
    lay = {}
    off = 0
    for name, nb in fields:
        lay[name] = (off, nb)
        off += (nb + 63) // 64 * 64
    return lay, off


def _g(v):
    """original node id -> padded global id"""
    return (v // NPC) * NPC_PAD + (v % NPC)


def preprocess(edge_index):
    """Returns (static, per_core) where static describes the shared program
    shape and per_core[c] holds the input blobs."""
    src = np.asarray(edge_index[0], dtype=np.int64)
    dst = np.asarray(edge_index[1], dtype=np.int64)
    loops = np.arange(N, dtype=np.int64)
    src = np.concatenate([src, loops])
    dst = np.concatenate([dst, loops])
    sp = _g(src)
    core = dst // NPC
    dloc_c = dst % NPC                      # 0..12499
    w = dloc_c // 128
    dloc_w = dloc_c % 128
    ch = sp // CHUNK
    srel = sp % CHUNK

    key = ((core * WINDOWS + w) * NCHUNK + ch).astype(np.int64)
    counts = np.bincount(key, minlength=CORES * WINDOWS * NCHUNK)
    counts = counts.reshape(CORES, WINDOWS, NCHUNK)
    maxc = counts.max(axis=0)               # [WINDOWS, NCHUNK]
    J = -(-maxc // 128)                     # ceil div; may be 0

    # superblocks
    sbs = [list(range(s, min(s + SW, WINDOWS))) for s in range(0, WINDOWS, SW)]

    # static slot layout per sb: chunk-major, then window
    sb_layout = []   # per sb: dict(ch -> [(w, slot_off_in_sb, J_w_ch)]), J_sb, per-window slot list
    for sb in sbs:
        off = 0
        per_ch = []
        win_slots = {ww: [] for ww in sb}
        for c in range(NCHUNK):
            groups = []
            for ww in sb:
                j = int(J[ww, c])
                if j == 0:
                    continue
                groups.append((ww, off, j))
                win_slots[ww].extend(range(off, off + j))
                off += j
            per_ch.append(groups)
        sb_layout.append(dict(per_ch=per_ch, J_sb=off, win_slots=win_slots))

    # static column offsets of each (sb, chunk) block in the resident idx tile
    idx_colo = []
    tot_cols = 0
    for si, sb in enumerate(sbs):
        cc = []
        for cidx in range(NCHUNK):
            groups = sb_layout[si]["per_ch"][cidx]
            Jch = sum(j for (_, _, j) in groups)
            cc.append(tot_cols)
            tot_cols += 8 * Jch             # (128*Jch)/16 columns
        idx_colo.append(cc)

    order = np.lexsort((srel, ch, w, core))
    so, wo, cho, srelo, dlwo, dlco = (
        x[order] for x in (sp, w, ch, srel, dloc_w, dloc_c))
    coreo = core[order]
    # group start offsets in sorted array per (core, w, ch)
    keyo = ((coreo * WINDOWS + wo) * NCHUNK + cho)
    starts = np.searchsorted(keyo, np.arange(CORES * WINDOWS * NCHUNK))
    ends = np.searchsorted(keyo, np.arange(CORES * WINDOWS * NCHUNK) + 1)

    per_core = []
    for c in range(CORES):
        idx_cols = []      # [16, cols] blocks, horizontally concatenated
        ad_blob = []
        dl_blob = []
        for si, sb in enumerate(sbs):
            lay = sb_layout[si]
            J_sb = lay["J_sb"]
            dl_arr = np.full((128, J_sb), 255, np.uint8)
            for cidx in range(NCHUNK):
                groups = lay["per_ch"][cidx]
                if not groups:
                    continue
                G = 128 * sum(j for (_, _, j) in groups)
                idx_flat = np.zeros(G, np.int16)
                off0 = groups[0][1]
                for (ww, soff, j) in groups:
                    gi = (c * WINDOWS + ww) * NCHUNK + cidx
                    s0, s1 = int(starts[gi]), int(ends[gi])
                    n = s1 - s0
                    gbase = (soff - off0) * 128
                    idx_flat[gbase:gbase + n] = srelo[s0:s1].astype(np.int16)
                    k = np.arange(n)
                    dl_arr[k % 128, soff + k // 128] = dlwo[s0:s1]
                idx_cols.append(idx_flat.reshape(G // 16, 16).T)  # [16, G/16]
            ad_blob.append(dl_arr.T.ravel())   # dstlocT flat [J_sb*128] u8
            dl_blob.append(dl_arr.ravel())
        per_core.append(dict(
            srcidx=np.ascontiguousarray(np.concatenate(idx_cols, axis=1)),
            dstloct=np.concatenate(ad_blob),
            dstloc=np.concatenate(dl_blob),
        ))
    static = dict(J=J, sbs=sbs, sb_layout=sb_layout, idx_colo=idx_colo,
                  tot_cols=tot_cols)
    return static, per_core


def build_program(static, blob_sizes):
    import os
    mode = os.environ.get("KMODE", "full")
    import concourse.bass as bass
    import concourse.bacc as bacc
    import concourse.tile as tile
    from concourse import mybir

    F32, BF, I16 = mybir.dt.float32, mybir.dt.bfloat16, mybir.dt.int16
    U8, I32, I8 = mybir.dt.uint8, mybir.dt.int32, mybir.dt.int8
    FEAT = I8 if UT_I8 else BF
    AF = mybir.ActivationFunctionType
    OP = mybir.AluOpType
    sbs, lay = static["sbs"], static["sb_layout"]
    idx_colo, TOT_COLS = static["idx_colo"], static["tot_cols"]

    reps = int(os.environ.get("KREPS", "1"))
    nc = bacc.Bacc("TRN2", target_bir_lowering=False, debug=False)
    P = nc.declare_dram_parameter
    LAYT, BLOBN = _layout(TOT_COLS, blob_sizes["ad"], blob_sizes["dl"])
    blob = P("blob", [BLOBN], U8, isOutput=False)
    out_ext = P("out", [1, NPC_PAD], F32, isOutput=True)

    def fld(name, dt=None, cols=None):
        off, nb = LAYT[name]
        ap = blob[off:off + nb]
        if dt is not None:
            ap = ap.bitcast(dt)
        if cols is not None:
            ap = ap.rearrange("(p s) -> p s", s=cols)
        return ap

    ut = fld("ut", FEAT, NPC_PAD)
    postt = fld("postt", FEAT, NPC_PAD)
    srcidx = fld("srcidx", I16, TOT_COLS)
    dstloct = fld("dstloct")
    dstloc = fld("dstloc")

    with tile.TileContext(nc) as tc:
        with (
            tc.tile_pool(name="cst", bufs=1) as cst,
            tc.tile_pool(name="sb", bufs=3) as sbp,
            tc.tile_pool(name="ps", bufs=2, space="PSUM") as psp,
            tc.tile_pool(name="dr", bufs=1, space="DRAM") as dr,
        ):
            tab1_shard = dr.tile([NPC_PAD, D1], BF)
            adtab1 = dr.tile([NPC_PAD, DAD], BF)
            tab2_shard = dr.tile([NPC_PAD, D2], BF)
            adtab2 = dr.tile([NPC_PAD, DAD], BF)
            x1t_dram = dr.tile([128, NPC_PAD], BF)

            w1a_sb = cst.tile([128, 140], BF)
            w2a_sb = cst.tile([128, 35], BF)
            fc1w_sb = cst.tile([96, 32], BF)
            fc2w_sb = cst.tile([32, 1], BF)
            fc1b_sb = cst.tile([32, 1], F32)
            fc2b_sb = cst.tile([1, 1], F32)
            b1rep_sb = cst.tile([128, 128], F32)
            b2rep_sb = cst.tile([128, 32], F32)
            for t, name, cols in [(w1a_sb, "w1a", 140), (w2a_sb, "w2a", 35),
                                  (fc1w_sb, "fc1w", 32), (fc2w_sb, "fc2w", 1),
                                  (fc1b_sb, "fc1b", 1), (fc2b_sb, "fc2b", 1)]:
                dt = F32 if name in ("fc1b", "fc2b") else BF
                nc.sync.dma_start(out=t[:], in_=fld(name, dt, cols))
            nc.sync.dma_start(
                out=b1rep_sb[:],
                in_=fld("b1row", F32)[None, :].to_broadcast([128, 128]))
            nc.sync.dma_start(
                out=b2rep_sb[:],
                in_=fld("b2row", F32)[None, :].to_broadcast([128, 32]))

            # on-device constants: iota row/col, identities, ones
            iotar_i = cst.tile([128, 128], I16)
            nc.gpsimd.iota(iotar_i[:], pattern=[[1, 128]], channel_multiplier=0)
            iota_sb = cst.tile([128, 128], BF)
            nc.vector.tensor_copy(out=iota_sb[:], in_=iotar_i[:])
            iotac_i = cst.tile([128, 1], I32)
            nc.gpsimd.iota(iotac_i[:], pattern=[[0, 1]], channel_multiplier=1)
            iotacol_sb = cst.tile([128, 1], F32)
            nc.vector.tensor_copy(out=iotacol_sb[:], in_=iotac_i[:])
            identbf_sb = cst.tile([128, 128], BF)
            nc.vector.tensor_scalar(
                out=identbf_sb[:], in0=iota_sb[:], scalar1=iotacol_sb[:, 0:1],
                scalar2=None, op0=OP.is_equal)
            identf_sb = cst.tile([128, 128], F32)
            nc.vector.tensor_scalar(
                out=identf_sb[:], in0=iota_sb[:], scalar1=iotacol_sb[:, 0:1],
                scalar2=None, op0=OP.is_equal)
            ones4_sb = cst.tile([128, 4], BF)
            nc.vector.memset(ones4_sb[:], 1.0)

            for _rep in range(reps):
                tab1_full = dr.tile([N_PAD, D1], BF, addr_space="Shared",
                                    name=f"tab1_full_r{_rep}")
                tab2_full = dr.tile([N_PAD, D2], BF, addr_space="Shared",
                                    name=f"tab2_full_r{_rep}")
                # ---- node phase 1: tables for layer 1 ----
                for t in range(WINDOWS if mode != "min" else 0):
                    sl = slice(t * 128, (t + 1) * 128)
                    if UT_I8:
                        lh8 = sbp.tile([128, 128], I8, tag="lh8")
                        nc.sync.dma_start(out=lh8[:], in_=ut[:, sl])
                        lh = sbp.tile([128, 128], BF, tag="lh")
                        nc.vector.tensor_copy(out=lh[:], in_=lh8[:])
                    else:
                        lh = sbp.tile([128, 128], BF, tag="lh")
                        nc.sync.dma_start(out=lh[:], in_=ut[:, sl])
                    acc = psp.tile([128, 140], F32, tag="acc", space="PSUM")
                    nc.tensor.matmul(out=acc[:], lhsT=lh[:], rhs=w1a_sb[:],
                                     start=True, stop=True)
                    rec = sbp.tile([128, D1], BF, tag="nrec")
                    nc.vector.tensor_copy(out=rec[:, 0:136], in_=acc[:, 0:136])
                    nc.vector.tensor_copy(
                        out=rec[:, 0:132].rearrange("p (h f) -> p h f", f=33)[:, :, 32],
                        in_=ones4_sb[:])
                    nc.sync.dma_start(out=tab1_shard[sl, :], in_=rec[:])
                    ad4 = sbp.tile([128, 4], BF, tag="ad4")
                    nc.vector.tensor_copy(out=ad4[:], in_=acc[:, 136:140])
                    nc.sync.dma_start(out=adtab1[sl, 0:4], in_=ad4[:])

                if mode not in ("noag", "min"):
                    nc.gpsimd.collective_compute(
                        "AllGather", mybir.AluOpType.bypass,
                        ins=[tab1_shard[:].opt()], outs=[tab1_full[:].opt()],
                        replica_groups=[list(range(CORES))])

                # ---- generic edge phase ----
                def edge_phase(tabfull, adtab, elem, H, mcols, epilogue, blob_offs):
                    ao, do = blob_offs
                    for si, sb in enumerate(sbs):
                        layd = lay[si]
                        J_sb = layd["J_sb"]
                        c0 = idx_colo[si][0]
                        c1 = (idx_colo[si + 1][0] if si + 1 < len(sbs)
                              else TOT_COLS)
                        sbcols = c1 - c0
                        idxt = sbp.tile([128, sbcols], I16, tag="idxt", bufs=2)
                        for r in range(8):
                            nc.sync.dma_start(
                                out=idxt[16 * r:16 * r + 16, :],
                                in_=srcidx[:, c0:c1])
                        rec = sbp.tile([128, J_sb * elem], BF, tag="erec", bufs=2)
                        for cidx in range(NCHUNK):
                            groups = layd["per_ch"][cidx]
                            if not groups:
                                continue
                            Jch = sum(j for (_, _, j) in groups)
                            off0 = groups[0][1]
                            G = 128 * Jch
                            if mode in ("nogather",):
                                continue
                            nc.gpsimd.dma_gather(
                                out_ap=rec[:, off0 * elem:(off0 + Jch) * elem]
                                    .rearrange("p (j d) -> p j d", d=elem),
                                in_ap=tabfull[cidx * CHUNK:(cidx + 1) * CHUNK, :],
                                idxs_ap=idxt[:, idx_colo[si][cidx] - c0:
                                             idx_colo[si][cidx] - c0 + 8 * Jch],
                                num_idxs=G, num_idxs_reg=G,
                                elem_size=elem, single_packet=False)
                        Gad = J_sb * 128
                        dtr8 = sbp.tile([128, Gad], U8, tag="adE8", bufs=2)
                        nc.sync.dma_start(
                            out=dtr8[:],
                            in_=dstloct[ao:ao + Gad][None, :].to_broadcast([128, Gad]))
                        ao += Gad
                        dtr = sbp.tile([128, Gad], BF, tag="adE", bufs=2)
                        nc.vector.tensor_copy(out=dtr[:], in_=dtr8[:])
                        ohT = sbp.tile([128, Gad], BF, tag="ohT", bufs=2)
                        nc.vector.tensor_scalar(
                            out=ohT[:], in0=dtr[:], scalar1=iotacol_sb[:, 0:1],
                            scalar2=None, op0=OP.is_equal)
                        adp = psp.tile([128, J_sb * H], F32, tag="adp", space="PSUM")
                        for ww2 in sb:
                            adw = sbp.tile([128, H], BF, tag="adw")
                            nc.sync.dma_start(
                                out=adw[:], in_=adtab[ww2 * 128:(ww2 + 1) * 128, 0:H])
                            for s_ in layd["win_slots"][ww2]:
                                nc.tensor.matmul(
                                    out=adp[:, s_ * H:(s_ + 1) * H],
                                    lhsT=ohT[:, s_ * 128:(s_ + 1) * 128],
                                    rhs=adw[:], start=True, stop=True)
                        dl8 = sbp.tile([128, J_sb], U8, tag="dl8")
                        nc.sync.dma_start(
                            out=dl8[:],
                            in_=dstloc[do:do + 128 * J_sb].rearrange(
                                "(p s) -> p s", s=J_sb))
                        do += 128 * J_sb
                        dl = sbp.tile([128, J_sb], BF, tag="dl")
                        nc.vector.tensor_copy(out=dl[:], in_=dl8[:])

                        if mode == "nocompute":
                            continue
                        recv = rec[:].rearrange("p (j d) -> p j d", d=elem)
                        adc = sbp.tile([128, J_sb * H], BF, tag="adc")
                        nc.vector.tensor_copy(out=adc[:], in_=adp[:])
                        e1 = sbp.tile([128, J_sb * H], F32, tag="e1")
                        nc.vector.tensor_tensor(
                            out=e1[:].rearrange("p (j h) -> p j h", h=H),
                            in0=recv[:, :, mcols:mcols + H],
                            in1=adc[:].rearrange("p (j h) -> p j h", h=H),
                            op=OP.add)
                        lr = sbp.tile([128, J_sb * H], F32, tag="lr")
                        nc.vector.tensor_scalar_mul(out=lr[:], in0=e1[:], scalar1=NEG)
                        nc.vector.tensor_tensor(out=e1[:], in0=e1[:], in1=lr[:], op=OP.max)
                        wgt = sbp.tile([128, J_sb * H], BF, tag="wgt")
                        nc.scalar.activation(out=wgt[:], in_=e1[:], func=AF.Exp)
                        msg = sbp.tile([128, J_sb * mcols], BF, tag="msg", bufs=2)
                        nc.vector.tensor_tensor(
                            out=msg[:].rearrange("p (j h f) -> p j h f", h=H, f=mcols // H),
                            in0=recv[:, :, 0:mcols].rearrange(
                                "p j (h f) -> p j h f", f=mcols // H),
                            in1=wgt[:].rearrange("p (j h) -> p j h", h=H)[:, :, :, None]
                                .to_broadcast([128, J_sb, H, mcols // H]),
                            op=OP.mult)
                        oh = sbp.tile([128, J_sb * 128], BF, tag="oh", bufs=2)
                        nc.vector.tensor_tensor(
                            out=oh[:].rearrange("p (j f) -> p j f", f=128),
                            in0=iota_sb[:][:, None, :].to_broadcast([128, J_sb, 128]),
                            in1=dl[:][:, :, None].to_broadcast([128, J_sb, 128]),
                            op=OP.is_equal)
                        for ww in sb:
                            slots = layd["win_slots"][ww]
                            if not slots:
                                continue
                            acc = psp.tile([128, mcols], F32, tag="acc", space="PSUM")
                            for i, s in enumerate(slots):
                                nc.tensor.matmul(
                                    out=acc[:],
                                    lhsT=oh[:, s * 128:(s + 1) * 128],
                                    rhs=msg[:, s * mcols:(s + 1) * mcols],
                                    start=(i == 0), stop=(i == len(slots) - 1))
                            epilogue(ww, acc)

                # ---- layer 1 epilogue ----
                def epi1(ww, acc):
                    den = sbp.tile([128, 4], F32, tag="den")
                    nc.vector.tensor_copy(
                        out=den[:],
                        in_=acc[:].rearrange("p (h f) -> p h f", f=33)[:, :, 32])
                    nc.vector.tensor_scalar_max(out=den[:], in0=den[:], scalar1=1e-30)
                    rcp = sbp.tile([128, 4], F32, tag="rcp")
                    nc.vector.reciprocal(out=rcp[:], in_=den[:])
                    x1 = sbp.tile([128, 128], F32, tag="x1")
                    accv = acc[:].rearrange("p (h f) -> p h f", f=33)
                    for h in range(HEADS):
                        nc.vector.tensor_scalar(
                            out=x1[:, h * 32:(h + 1) * 32],
                            in0=accv[:, h, 0:32],
                            scalar1=rcp[:, h:h + 1], scalar2=None, op0=OP.mult)
                    nc.vector.tensor_tensor(out=x1[:], in0=x1[:], in1=b1rep_sb[:], op=OP.add)
                    x1b = sbp.tile([128, 128], BF, tag="x1b")
                    nc.scalar.activation(out=x1b[:], in_=x1[:], func=AF.Relu)
                    tp = psp.tile([128, 128], BF, tag="tp", space="PSUM")
                    nc.tensor.transpose(out=tp[:], in_=x1b[:], identity=identbf_sb[:])
                    x1t = sbp.tile([128, 128], BF, tag="x1t")
                    nc.vector.tensor_copy(out=x1t[:], in_=tp[:])
                    nc.sync.dma_start(
                        out=x1t_dram[:, ww * 128:(ww + 1) * 128], in_=x1t[:])

                if mode not in ("noedge", "noag", "min"):
                    edge_phase(tab1_full, adtab1, D1, HEADS, 132, epi1, (0, 0))

                # ---- node phase 2 ----
                for t in range(WINDOWS if mode != "min" else 0):
                    sl = slice(t * 128, (t + 1) * 128)
                    lh2 = sbp.tile([128, 128], BF, tag="lh")
                    nc.sync.dma_start(out=lh2[:], in_=x1t_dram[:, sl])
                    acc = psp.tile([128, 35], F32, tag="acc", space="PSUM")
                    nc.tensor.matmul(out=acc[:], lhsT=lh2[:], rhs=w2a_sb[:],
                                     start=True, stop=True)
                    rec2 = sbp.tile([128, D2], BF, tag="nrec")
                    nc.vector.tensor_copy(out=rec2[:, 0:34], in_=acc[:, 0:34])
                    nc.vector.tensor_copy(out=rec2[:, 32:33], in_=ones4_sb[:, 0:1])
                    nc.sync.dma_start(out=tab2_shard[sl, :], in_=rec2[:])
                    ad1c = sbp.tile([128, 1], BF, tag="ad4")
                    nc.vector.tensor_copy(out=ad1c[:], in_=acc[:, 34:35])
                    nc.sync.dma_start(out=adtab2[sl, 0:1], in_=ad1c[:])

                if mode not in ("noag", "min"):
                    nc.gpsimd.collective_compute(
                        "AllGather", mybir.AluOpType.bypass,
                        ins=[tab2_shard[:].opt()], outs=[tab2_full[:].opt()],
                        replica_groups=[list(range(CORES))])

                # ---- layer 2 epilogue (+ fused FC head) ----
                def epi2(ww, acc):
                    den = sbp.tile([128, 1], F32, tag="den")
                    nc.vector.tensor_copy(out=den[:], in_=acc[:, 32:33])
                    nc.vector.tensor_scalar_max(out=den[:], in0=den[:], scalar1=1e-30)
                    rcp = sbp.tile([128, 1], F32, tag="rcp")
                    nc.vector.reciprocal(out=rcp[:], in_=den[:])
                    x2 = sbp.tile([128, 32], F32, tag="x2")
                    nc.vector.tensor_scalar(
                        out=x2[:], in0=acc[:, 0:32],
                        scalar1=rcp[:, 0:1], scalar2=None, op0=OP.mult)
                    nc.vector.tensor_tensor(out=x2[:], in0=x2[:], in1=b2rep_sb[:], op=OP.add)
                    x2f = sbp.tile([128, 32], F32, tag="x2f")
                    nc.scalar.activation(out=x2f[:], in_=x2[:], func=AF.Relu)
                    tp2 = psp.tile([32, 128], F32, tag="tp", space="PSUM")
                    nc.tensor.transpose(out=tp2[:], in_=x2f[:], identity=identf_sb[:])
                    # zt rows: [post 0:64 | x2T 64:96] (compute-engine APs must
                    # stay in an aligned partition subtree); fc1w rows match
                    zt = sbp.tile([96, 128], BF, tag="zt")
                    nc.vector.tensor_copy(out=zt[64:96, :], in_=tp2[:])
                    if UT_I8:
                        pt8 = sbp.tile([64, 128], I8, tag="pt8")
                        nc.sync.dma_start(out=pt8[:],
                                          in_=postt[:, ww * 128:(ww + 1) * 128])
                        nc.vector.tensor_copy(out=zt[0:64, :], in_=pt8[:])
                    else:
                        nc.sync.dma_start(out=zt[0:64, :],
                                          in_=postt[:, ww * 128:(ww + 1) * 128])
                    pa = psp.tile([32, 128], F32, tag="fc", space="PSUM")
                    nc.tensor.matmul(out=pa[:], lhsT=fc1w_sb[:], rhs=zt[:],
                                     start=True, stop=True)
                    y1 = sbp.tile([32, 128], BF, tag="y1")
                    nc.scalar.activation(out=y1[:], in_=pa[:], func=AF.Relu,
                                         bias=fc1b_sb[:])
                    pb = psp.tile([1, 128], F32, tag="fc", space="PSUM")
                    nc.tensor.matmul(out=pb[:], lhsT=fc2w_sb[:], rhs=y1[:],
                                     start=True, stop=True)
                    yo = sbp.tile([1, 128], F32, tag="yo")
                    nc.scalar.activation(out=yo[:], in_=pb[:], func=AF.Sigmoid,
                                         bias=fc2b_sb[:])
                    nc.sync.dma_start(out=out_ext[0:1, ww * 128:(ww + 1) * 128],
                                      in_=yo[:])

                if mode not in ("noedge", "noag", "min"):
                    edge_phase(tab2_full, adtab2, D2, 1, 33, epi2, (0, 0))
            if mode == "min":
                zo = sbp.tile([1, NPC_PAD], F32, tag="zo")
                nc.vector.memset(zo[:], 0.5)
                nc.sync.dma_start(out=out_ext[:], in_=zo[:])

    nc.compile()
    return nc


def _make_inputs(user_features, post_features, W1, a1s, a1d, b1,
                 W2, a2s, a2d, b2, fc1_w, fc1_b, fc2_w, fc2_b, per_core):
    uf = np.asarray(user_features, np.float32)
    pf = np.asarray(post_features, np.float32)
    W1 = np.asarray(W1, np.float32)
    W2 = np.asarray(W2, np.float32)
    a1s = np.asarray(a1s, np.float32)
    a1d = np.asarray(a1d, np.float32)
    a2s = np.asarray(a2s, np.float32)
    a2d = np.asarray(a2d, np.float32)

    w1a = np.zeros((128, 140), np.float32)
    for h in range(HEADS):
        w1a[:, h * 33:h * 33 + 32] = W1[:, h * 32:(h + 1) * 32]
        w1a[:, 132 + h] = W1[:, h * 32:(h + 1) * 32] @ a1s[h]
        w1a[:, 136 + h] = W1[:, h * 32:(h + 1) * 32] @ a1d[h]
    w2a = np.zeros((128, 35), np.float32)
    w2a[:, 0:32] = W2
    w2a[:, 33] = W2 @ a2s[0]
    w2a[:, 34] = W2 @ a2d[0]

    fc1_w = np.asarray(fc1_w, np.float32).copy()
    if UT_I8:
        # int8-quantize features; fold the dequant scales into the weights
        # (b1/fc biases are applied after aggregation, so this is exact)
        s_u = float(np.abs(uf).max()) / 127.0
        s_p = float(np.abs(pf).max()) / 127.0
        uf = np.clip(np.round(uf / s_u), -127, 127)
        pf = np.clip(np.round(pf / s_p), -127, 127)
        w1a *= s_u
        fc1_w[32:96, :] *= s_p
    # zt rows are [post | x2T], so reorder fc1w rows to match
    fc1_w = np.concatenate([fc1_w[32:96], fc1_w[0:32]], axis=0)
    FEAT_NP = np.int8 if UT_I8 else BF16

    lay, BLOBN = _layout(per_core[0]["srcidx"].shape[1],
                         len(per_core[0]["dstloct"]),
                         len(per_core[0]["dstloc"]))

    def pack(blob, name, arr):
        off, nb = lay[name]
        raw = np.ascontiguousarray(arr).view(np.uint8).ravel()
        assert raw.nbytes == nb, (name, raw.nbytes, nb)
        blob[off:off + nb] = raw

    consts = [
        ("w1a", w1a.astype(BF16)),
        ("w2a", w2a.astype(BF16)),
        ("fc1w", fc1_w.astype(BF16)),
        ("fc2w", np.asarray(fc2_w, np.float32).astype(BF16)),
        ("fc1b", np.asarray(fc1_b, np.float32).reshape(32, 1).copy()),
        ("fc2b", np.asarray(fc2_b, np.float32).reshape(1, 1).copy()),
        ("b1row", np.asarray(b1, np.float32).reshape(1, 128).copy()),
        ("b2row", np.asarray(b2, np.float32).reshape(1, 32).copy()),
    ]
    in_maps = []
    for c in range(CORES):
        sl = slice(c * NPC, (c + 1) * NPC)
        ut = np.zeros((128, NPC_PAD), np.float32)
        ut[:, :NPC] = uf[sl].T
        postt = np.zeros((F_POST, NPC_PAD), np.float32)
        postt[:, :NPC] = pf[sl].T
        blob = np.zeros(BLOBN, np.uint8)
        pack(blob, "ut", ut.astype(FEAT_NP))
        pack(blob, "postt", postt.astype(FEAT_NP))
        for name, arr in consts:
            pack(blob, name, arr)
        for name in ("srcidx", "dstloct", "dstloc"):
            pack(blob, name, per_core[c][name])
        in_maps.append(dict(blob=blob))
    return in_maps


_CACHE = {}
LAST_EXEC_NS = None


def kernel(**inputs):
    from concourse.bass_utils import run_bass_kernel_spmd
    ei = np.asarray(inputs["edge_index"])
    static, per_core = preprocess(ei)
    blob_sizes = dict(ad=len(per_core[0]["dstloct"]),
                      dl=len(per_core[0]["dstloc"]))
    in_maps = _make_inputs(
        inputs["user_features"], inputs["post_features"],
        inputs["W1"], inputs["a1s"], inputs["a1d"], inputs["b1"],
        inputs["W2"], inputs["a2s"], inputs["a2d"], inputs["b2"],
        inputs["fc1_w"], inputs["fc1_b"], inputs["fc2_w"], inputs["fc2_b"],
        per_core)
    key = (static["tot_cols"], blob_sizes["ad"], blob_sizes["dl"])
    if key not in _CACHE:
        _CACHE[key] = build_program(static, blob_sizes)
    nc = _CACHE[key]
    import os
    trace = bool(os.environ.get("BASS_KERNEL_TRACE"))
    r = run_bass_kernel_spmd(nc, in_maps, list(range(CORES)), trace=trace)
    global LAST_EXEC_NS
    LAST_EXEC_NS = r.exec_time_ns
    out = np.empty((N, 1), np.float32)
    for c in range(CORES):
        out[c * NPC:(c + 1) * NPC, 0] = r.results[c]["out"][0, :NPC]
    return out


# revision 13
# speedup vs baseline: 5.0075x; 1.2001x over previous
"""GAT model (2-layer GAT + FC head) on 8 Trainium2 NeuronCores.

Strategy: destination-sharded. Each core owns 12544 (padded) dst nodes
= 98 windows of 128. Edges live on their dst's core, sorted into
(window, src-chunk) groups. Node phase computes per-node tables
[h | as] (bf16) sharded + AllGather; ad values stay core-local.
Edge phase: dma_gather of 512B records by src (int16 idx over 4
chunks of 25088 rows) + broadcast of dst-local ids; per-edge
softmax weights w = exp(leakyrelu(as+ad)) (no segment-max needed:
scores are bounded, exp cannot overflow in f32); messages
msg = w * [h | 1] scattered into per-window PSUM via one-hot matmuls
(one-hot built in bulk on DVE from iota==dstloc). Denominator rides
the matmul via the record's ones-column. FC head fused per window.

Host->device upload is the wall-clock bottleneck (axon PJRT tunnel),
so inputs are minimized: gather indices uploaded unreplicated
[16, cols] and tiled 8x across partitions on device into one
resident SBUF tile; dst-local ids as uint8 (cast on device);
node/post features as scaled int8 (cast to bf16 on device,
scales folded into w1a/fc1w on host); iota /
identity / replicated-bias constants built on device.
"""
import sys
import numpy as np
import ml_dtypes

sys.path.insert(0, "/opt/trn_rl_repo")

# Persistent XLA compilation cache: run_bass_kernel_spmd rebuilds its jit
# closure every call, which otherwise re-compiles the (identical) wrapper
# executable each time (~1.1 s/call). With the cache, repeat calls
# deserialize instead.
try:
    import jax as _jax
    _jax.config.update("jax_compilation_cache_dir", "/tmp/jax_comp_cache")
    _jax.config.update("jax_persistent_cache_min_compile_time_secs", 0.0)
    _jax.config.update("jax_persistent_cache_min_entry_size_bytes", 0)
except Exception:
    pass

BF16 = ml_dtypes.bfloat16

N = 100000
E_RAW = 1600000
F_USER = 128
F_POST = 64
HID = 32
HEADS = 4
NEG = 0.2
CORES = 8
NPC = 12500                 # real nodes per core
NPC_PAD = 12544             # 98 * 128
WINDOWS = 98
N_PAD = NPC_PAD * CORES     # 100352
NCHUNK = 4
CHUNK = N_PAD // NCHUNK     # 25088
SW = 2                      # windows per superblock
D1 = 256                    # table1 row elems (bf16): [hblk 132 | as 4 | pad]
D2 = 128                    # table2 row elems: [h2blk 33 | as2 1 | pad]
DAD = 128                   # ad table row elems: [ad .. | pad]
UT_I8 = True                # upload node/post features as scaled int8


def _layout(tot_cols, adlen, dllen):
    """Byte layout of the single packed input blob (64B-aligned fields)."""
    fb = 1 if UT_I8 else 2
    fields = [
        ("srcidx", 16 * tot_cols * 2),
        ("ut", 128 * NPC_PAD * fb),
        ("postt", F_POST * NPC_PAD * fb),
        ("dstloct", adlen),
        ("dstloc", dllen),
        ("w1a", 128 * 140 * 2),
        ("w2a", 128 * 35 * 2),
        ("fc1w", 96 * 32 * 2),
        ("fc2w", 32 * 1 * 2),
        ("fc1b", 32 * 4),
        ("fc2b", 4),
        ("b1row", 128 * 4),
        ("b2row", 32 * 4),
    ]
    lay = {}
    off = 0
    for name, nb in fields:
        lay[name] = (off, nb)
        off += (nb + 63) // 64 * 64
    return lay, off


def _g(v):
    """original node id -> padded global id"""
    return (v // NPC) * NPC_PAD + (v % NPC)


def preprocess(edge_index):
    """Returns (static, per_core) where static describes the shared program
    shape and per_core[c] holds the input blobs."""
    src = np.asarray(edge_index[0], dtype=np.int64)
    dst = np.asarray(edge_index[1], dtype=np.int64)
    loops = np.arange(N, dtype=np.int64)
    src = np.concatenate([src, loops])
    dst = np.concatenate([dst, loops])
    sp = _g(src)
    core = dst // NPC
    dloc_c = dst % NPC                      # 0..12499
    w = dloc_c // 128
    dloc_w = dloc_c % 128
    ch = sp // CHUNK
    srel = sp % CHUNK

    key = ((core * WINDOWS + w) * NCHUNK + ch).astype(np.int64)
    counts = np.bincount(key, minlength=CORES * WINDOWS * NCHUNK)
    counts = counts.reshape(CORES, WINDOWS, NCHUNK)
    maxc = counts.max(axis=0)               # [WINDOWS, NCHUNK]
    J = -(-maxc // 128)                     # ceil div; may be 0

    # superblocks
    sbs = [list(range(s, min(s + SW, WINDOWS))) for s in range(0, WINDOWS, SW)]

    # static slot layout per sb: chunk-major, then window
    sb_layout = []   # per sb: dict(ch -> [(w, slot_off_in_sb, J_w_ch)]), J_sb, per-window slot list
    for sb in sbs:
        off = 0
        per_ch = []
        win_slots = {ww: [] for ww in sb}
        for c in range(NCHUNK):
            groups = []
            for ww in sb:
                j = int(J[ww, c])
                if j == 0:
                    continue
                groups.append((ww, off, j))
                win_slots[ww].extend(range(off, off + j))
                off += j
            per_ch.append(groups)
        sb_layout.append(dict(per_ch=per_ch, J_sb=off, win_slots=win_slots))

    # static column offsets of each (sb, chunk) block in the resident idx tile
    idx_colo = []
    tot_cols = 0
    for si, sb in enumerate(sbs):
        cc = []
        for cidx in range(NCHUNK):
            groups = sb_layout[si]["per_ch"][cidx]
            Jch = sum(j for (_, _, j) in groups)
            cc.append(tot_cols)
            tot_cols += 8 * Jch             # (128*Jch)/16 columns
        idx_colo.append(cc)

    order = np.lexsort((srel, ch, w, core))
    so, wo, cho, srelo, dlwo, dlco = (
        x[order] for x in (sp, w, ch, srel, dloc_w, dloc_c))
    coreo = core[order]
    # group start offsets in sorted array per (core, w, ch)
    keyo = ((coreo * WINDOWS + wo) * NCHUNK + cho)
    starts = np.searchsorted(keyo, np.arange(CORES * WINDOWS * NCHUNK))
    ends = np.searchsorted(keyo, np.arange(CORES * WINDOWS * NCHUNK) + 1)

    per_core = []
    for c in range(CORES):
        idx_cols = []      # [16, cols] blocks, horizontally concatenated
        ad_blob = []
        dl_blob = []
        for si, sb in enumerate(sbs):
            lay = sb_layout[si]
            J_sb = lay["J_sb"]
            dl_arr = np.full((128, J_sb), 255, np.uint8)
            for cidx in range(NCHUNK):
                groups = lay["per_ch"][cidx]
                if not groups:
                    continue
                G = 128 * sum(j for (_, _, j) in groups)
                idx_flat = np.zeros(G, np.int16)
                off0 = groups[0][1]
                for (ww, soff, j) in groups:
                    gi = (c * WINDOWS + ww) * NCHUNK + cidx
                    s0, s1 = int(starts[gi]), int(ends[gi])
                    n = s1 - s0
                    gbase = (soff - off0) * 128
                    idx_flat[gbase:gbase + n] = srelo[s0:s1].astype(np.int16)
                    k = np.arange(n)
                    dl_arr[k % 128, soff + k // 128] = dlwo[s0:s1]
                idx_cols.append(idx_flat.reshape(G // 16, 16).T)  # [16, G/16]
            ad_blob.append(dl_arr.T.ravel())   # dstlocT flat [J_sb*128] u8
            dl_blob.append(dl_arr.ravel())
        per_core.append(dict(
            srcidx=np.ascontiguousarray(np.concatenate(idx_cols, axis=1)),
            dstloct=np.concatenate(ad_blob),
            dstloc=np.concatenate(dl_blob),
        ))
    static = dict(J=J, sbs=sbs, sb_layout=sb_layout, idx_colo=idx_colo,
                  tot_cols=tot_cols)
    return static, per_core


def build_program(static, blob_sizes):
    import os
    mode = os.environ.get("KMODE", "full")
    import concourse.bass as bass
    import concourse.bacc as bacc
    import concourse.tile as tile
    from concourse import mybir

    F32, BF, I16 = mybir.dt.float32, mybir.dt.bfloat16, mybir.dt.int16
    U8, I32, I8 = mybir.dt.uint8, mybir.dt.int32, mybir.dt.int8
    FEAT = I8 if UT_I8 else BF
    AF = mybir.ActivationFunctionType
    OP = mybir.AluOpType
    sbs, lay = static["sbs"], static["sb_layout"]
    idx_colo, TOT_COLS = static["idx_colo"], static["tot_cols"]

    reps = int(os.environ.get("KREPS", "1"))
    nc = bacc.Bacc("TRN2", target_bir_lowering=False, debug=False)
    P = nc.declare_dram_parameter
    LAYT, BLOBN = _layout(TOT_COLS, blob_sizes["ad"], blob_sizes["dl"])
    blob = P("blob", [BLOBN], U8, isOutput=False)
    out_ext = P("out", [1, NPC_PAD], F32, isOutput=True)

    def fld(name, dt=None, cols=None):
        off, nb = LAYT[name]
        ap = blob[off:off + nb]
        if dt is not None:
            ap = ap.bitcast(dt)
        if cols is not None:
            ap = ap.rearrange("(p s) -> p s", s=cols)
        return ap

    ut = fld("ut", FEAT, NPC_PAD)
    postt = fld("postt", FEAT, NPC_PAD)
    srcidx = fld("srcidx", I16, TOT_COLS)
    dstloct = fld("dstloct")
    dstloc = fld("dstloc")

    with tile.TileContext(nc) as tc:
        with (
            tc.tile_pool(name="cst", bufs=1) as cst,
            tc.tile_pool(name="sb", bufs=3) as sbp,
            tc.tile_pool(name="ps", bufs=2, space="PSUM") as psp,
            tc.tile_pool(name="dr", bufs=1, space="DRAM") as dr,
        ):
            tab1_shard = dr.tile([NPC_PAD, D1], BF)
            adtab1 = dr.tile([NPC_PAD, DAD], BF)
            tab2_shard = dr.tile([NPC_PAD, D2], BF)
            adtab2 = dr.tile([NPC_PAD, DAD], BF)
            x1t_dram = dr.tile([128, NPC_PAD], BF)

            w1a_sb = cst.tile([128, 140], BF)
            w2a_sb = cst.tile([128, 35], BF)
            fc1w_sb = cst.tile([96, 32], BF)
            fc2w_sb = cst.tile([32, 1], BF)
            fc1b_sb = cst.tile([32, 1], F32)
            fc2b_sb = cst.tile([1, 1], F32)
            b1rep_sb = cst.tile([128, 128], F32)
            b2rep_sb = cst.tile([128, 32], F32)
            for t, name, cols in [(w1a_sb, "w1a", 140), (w2a_sb, "w2a", 35),
                                  (fc1w_sb, "fc1w", 32), (fc2w_sb, "fc2w", 1),
                                  (fc1b_sb, "fc1b", 1), (fc2b_sb, "fc2b", 1)]:
                dt = F32 if name in ("fc1b", "fc2b") else BF
                nc.sync.dma_start(out=t[:], in_=fld(name, dt, cols))
            nc.sync.dma_start(
                out=b1rep_sb[:],
                in_=fld("b1row", F32)[None, :].to_broadcast([128, 128]))
            nc.sync.dma_start(
                out=b2rep_sb[:],
                in_=fld("b2row", F32)[None, :].to_broadcast([128, 32]))

            # on-device constants: iota row/col, identities, ones
            iotar_i = cst.tile([128, 128], I16)
            nc.gpsimd.iota(iotar_i[:], pattern=[[1, 128]], channel_multiplier=0)
            iota_sb = cst.tile([128, 128], BF)
            nc.vector.tensor_copy(out=iota_sb[:], in_=iotar_i[:])
            iotac_i = cst.tile([128, 1], I32)
            nc.gpsimd.iota(iotac_i[:], pattern=[[0, 1]], channel_multiplier=1)
            iotacol_sb = cst.tile([128, 1], F32)
            nc.vector.tensor_copy(out=iotacol_sb[:], in_=iotac_i[:])
            identbf_sb = cst.tile([128, 128], BF)
            nc.vector.tensor_scalar(
                out=identbf_sb[:], in0=iota_sb[:], scalar1=iotacol_sb[:, 0:1],
                scalar2=None, op0=OP.is_equal)
            identf_sb = cst.tile([128, 128], F32)
            nc.vector.tensor_scalar(
                out=identf_sb[:], in0=iota_sb[:], scalar1=iotacol_sb[:, 0:1],
                scalar2=None, op0=OP.is_equal)
            ones4_sb = cst.tile([128, 4], BF)
            nc.vector.memset(ones4_sb[:], 1.0)

            for _rep in range(reps):
                tab1_full = dr.tile([N_PAD, D1], BF, addr_space="Shared",
                                    name=f"tab1_full_r{_rep}")
                tab2_full = dr.tile([N_PAD, D2], BF, addr_space="Shared",
                                    name=f"tab2_full_r{_rep}")
                # ---- node phase 1: tables for layer 1 ----
                for t in range(WINDOWS if mode != "min" else 0):
                    sl = slice(t * 128, (t + 1) * 128)
                    if UT_I8:
                        lh8 = sbp.tile([128, 128], I8, tag="lh8")
                        nc.sync.dma_start(out=lh8[:], in_=ut[:, sl])
                        lh = sbp.tile([128, 128], BF, tag="lh")
                        nc.vector.tensor_copy(out=lh[:], in_=lh8[:])
                    else:
                        lh = sbp.tile([128, 128], BF, tag="lh")
                        nc.sync.dma_start(out=lh[:], in_=ut[:, sl])
                    acc = psp.tile([128, 140], F32, tag="acc", space="PSUM")
                    nc.tensor.matmul(out=acc[:], lhsT=lh[:], rhs=w1a_sb[:],
                                     start=True, stop=True)
                    rec = sbp.tile([128, D1], BF, tag="nrec")
                    nc.vector.tensor_copy(out=rec[:, 0:136], in_=acc[:, 0:136])
                    nc.vector.tensor_copy(
                        out=rec[:, 0:132].rearrange("p (h f) -> p h f", f=33)[:, :, 32],
                        in_=ones4_sb[:])
                    nc.sync.dma_start(out=tab1_shard[sl, :], in_=rec[:])
                    ad4 = sbp.tile([128, 4], BF, tag="ad4")
                    nc.vector.tensor_copy(out=ad4[:], in_=acc[:, 136:140])
                    nc.sync.dma_start(out=adtab1[sl, 0:4], in_=ad4[:])

                if mode not in ("noag", "min"):
                    nc.gpsimd.collective_compute(
                        "AllGather", mybir.AluOpType.bypass,
                        ins=[tab1_shard[:].opt()], outs=[tab1_full[:].opt()],
                        replica_groups=[list(range(CORES))])

                # ---- generic edge phase ----
                def edge_phase(tabfull, adtab, elem, H, mcols, epilogue, blob_offs):
                    ao, do = blob_offs
                    for si, sb in enumerate(sbs):
                        layd = lay[si]
                        J_sb = layd["J_sb"]
                        c0 = idx_colo[si][0]
                        c1 = (idx_colo[si + 1][0] if si + 1 < len(sbs)
                              else TOT_COLS)
                        sbcols = c1 - c0
                        idxt = sbp.tile([128, sbcols], I16, tag="idxt", bufs=2)
                        for r in range(8):
                            nc.sync.dma_start(
                                out=idxt[16 * r:16 * r + 16, :],
                                in_=srcidx[:, c0:c1])
                        rec = sbp.tile([128, J_sb * elem], BF, tag="erec", bufs=2)
                        for cidx in range(NCHUNK):
                            groups = layd["per_ch"][cidx]
                            if not groups:
                                continue
                            Jch = sum(j for (_, _, j) in groups)
                            off0 = groups[0][1]
                            G = 128 * Jch
                            if mode in ("nogather",):
                                continue
                            nc.gpsimd.dma_gather(
                                out_ap=rec[:, off0 * elem:(off0 + Jch) * elem]
                                    .rearrange("p (j d) -> p j d", d=elem),
                                in_ap=tabfull[cidx * CHUNK:(cidx + 1) * CHUNK, :],
                                idxs_ap=idxt[:, idx_colo[si][cidx] - c0:
                                             idx_colo[si][cidx] - c0 + 8 * Jch],
                                num_idxs=G, num_idxs_reg=G,
                                elem_size=elem, single_packet=False)
                        Gad = J_sb * 128
                        dtr8 = sbp.tile([128, Gad], U8, tag="adE8", bufs=2)
                        nc.sync.dma_start(
                            out=dtr8[:],
                            in_=dstloct[ao:ao + Gad][None, :].to_broadcast([128, Gad]))
                        ao += Gad
                        dtr = sbp.tile([128, Gad], BF, tag="adE", bufs=2)
                        nc.vector.tensor_copy(out=dtr[:], in_=dtr8[:])
                        ohT = sbp.tile([128, Gad], BF, tag="ohT", bufs=2)
                        nc.vector.tensor_scalar(
                            out=ohT[:], in0=dtr[:], scalar1=iotacol_sb[:, 0:1],
                            scalar2=None, op0=OP.is_equal)
                        adp = psp.tile([128, J_sb * H], F32, tag="adp", space="PSUM")
                        for ww2 in sb:
                            adw = sbp.tile([128, H], BF, tag="adw")
                            nc.sync.dma_start(
                                out=adw[:], in_=adtab[ww2 * 128:(ww2 + 1) * 128, 0:H])
                            for s_ in layd["win_slots"][ww2]:
                                nc.tensor.matmul(
                                    out=adp[:, s_ * H:(s_ + 1) * H],
                                    lhsT=ohT[:, s_ * 128:(s_ + 1) * 128],
                                    rhs=adw[:], start=True, stop=True)
                        dl8 = sbp.tile([128, J_sb], U8, tag="dl8")
                        nc.sync.dma_start(
                            out=dl8[:],
                            in_=dstloc[do:do + 128 * J_sb].rearrange(
                                "(p s) -> p s", s=J_sb))
                        do += 128 * J_sb
                        dl = sbp.tile([128, J_sb], BF, tag="dl")
                        nc.vector.tensor_copy(out=dl[:], in_=dl8[:])

                        if mode == "nocompute":
                            continue
                        recv = rec[:].rearrange("p (j d) -> p j d", d=elem)
                        adc = sbp.tile([128, J_sb * H], BF, tag="adc")
                        nc.vector.tensor_copy(out=adc[:], in_=adp[:])
                        e1 = sbp.tile([128, J_sb * H], F32, tag="e1")
                        nc.vector.tensor_tensor(
                            out=e1[:].rearrange("p (j h) -> p j h", h=H),
                            in0=recv[:, :, mcols:mcols + H],
                            in1=adc[:].rearrange("p (j h) -> p j h", h=H),
                            op=OP.add)
                        lr = sbp.tile([128, J_sb * H], F32, tag="lr")
                        nc.vector.tensor_scalar_mul(out=lr[:], in0=e1[:], scalar1=NEG)
                        nc.vector.tensor_tensor(out=e1[:], in0=e1[:], in1=lr[:], op=OP.max)
                        wgt = sbp.tile([128, J_sb * H], BF, tag="wgt")
                        nc.scalar.activation(out=wgt[:], in_=e1[:], func=AF.Exp)
                        msg = sbp.tile([128, J_sb * mcols], BF, tag="msg", bufs=2)
                        nc.vector.tensor_tensor(
                            out=msg[:].rearrange("p (j h f) -> p j h f", h=H, f=mcols // H),
                            in0=recv[:, :, 0:mcols].rearrange(
                                "p j (h f) -> p j h f", f=mcols // H),
                            in1=wgt[:].rearrange("p (j h) -> p j h", h=H)[:, :, :, None]
                                .to_broadcast([128, J_sb, H, mcols // H]),
                            op=OP.mult)
                        oh = sbp.tile([128, J_sb * 128], BF, tag="oh", bufs=2)
                        nc.vector.tensor_tensor(
                            out=oh[:].rearrange("p (j f) -> p j f", f=128),
                            in0=iota_sb[:][:, None, :].to_broadcast([128, J_sb, 128]),
                            in1=dl[:][:, :, None].to_broadcast([128, J_sb, 128]),
                            op=OP.is_equal)
                        for ww in sb:
                            slots = layd["win_slots"][ww]
                            if not slots:
                                continue
                            acc = psp.tile([128, mcols], F32, tag="acc", space="PSUM")
                            for i, s in enumerate(slots):
                                nc.tensor.matmul(
                                    out=acc[:],
                                    lhsT=oh[:, s * 128:(s + 1) * 128],
                                    rhs=msg[:, s * mcols:(s + 1) * mcols],
                                    start=(i == 0), stop=(i == len(slots) - 1))
                            epilogue(ww, acc)

                # ---- layer 1 epilogue ----
                def epi1(ww, acc):
                    den = sbp.tile([128, 4], F32, tag="den")
                    nc.vector.tensor_copy(
                        out=den[:],
                        in_=acc[:].rearrange("p (h f) -> p h f", f=33)[:, :, 32])
                    nc.vector.tensor_scalar_max(out=den[:], in0=den[:], scalar1=1e-30)
                    rcp = sbp.tile([128, 4], F32, tag="rcp")
                    nc.vector.reciprocal(out=rcp[:], in_=den[:])
                    x1 = sbp.tile([128, 128], F32, tag="x1")
                    accv = acc[:].rearrange("p (h f) -> p h f", f=33)
                    for h in range(HEADS):
                        nc.vector.tensor_scalar(
                            out=x1[:, h * 32:(h + 1) * 32],
                            in0=accv[:, h, 0:32],
                            scalar1=rcp[:, h:h + 1], scalar2=None, op0=OP.mult)
                    nc.vector.tensor_tensor(out=x1[:], in0=x1[:], in1=b1rep_sb[:], op=OP.add)
                    x1b = sbp.tile([128, 128], BF, tag="x1b")
                    nc.scalar.activation(out=x1b[:], in_=x1[:], func=AF.Relu)
                    tp = psp.tile([128, 128], BF, tag="tp", space="PSUM")
                    nc.tensor.transpose(out=tp[:], in_=x1b[:], identity=identbf_sb[:])
                    x1t = sbp.tile([128, 128], BF, tag="x1t")
                    nc.vector.tensor_copy(out=x1t[:], in_=tp[:])
                    nc.sync.dma_start(
                        out=x1t_dram[:, ww * 128:(ww + 1) * 128], in_=x1t[:])

                if mode not in ("noedge", "noag", "min"):
                    edge_phase(tab1_full, adtab1, D1, HEADS, 132, epi1, (0, 0))

                # ---- node phase 2 ----
                for t in range(WINDOWS if mode != "min" else 0):
                    sl = slice(t * 128, (t + 1) * 128)
                    lh2 = sbp.tile([128, 128], BF, tag="lh")
                    nc.sync.dma_start(out=lh2[:], in_=x1t_dram[:, sl])
                    acc = psp.tile([128, 35], F32, tag="acc", space="PSUM")
                    nc.tensor.matmul(out=acc[:], lhsT=lh2[:], rhs=w2a_sb[:],
                                     start=True, stop=True)
                    rec2 = sbp.tile([128, D2], BF, tag="nrec")
                    nc.vector.tensor_copy(out=rec2[:, 0:34], in_=acc[:, 0:34])
                    nc.vector.tensor_copy(out=rec2[:, 32:33], in_=ones4_sb[:, 0:1])
                    nc.sync.dma_start(out=tab2_shard[sl, :], in_=rec2[:])
                    ad1c = sbp.tile([128, 1], BF, tag="ad4")
                    nc.vector.tensor_copy(out=ad1c[:], in_=acc[:, 34:35])
                    nc.sync.dma_start(out=adtab2[sl, 0:1], in_=ad1c[:])

                if mode not in ("noag", "min"):
                    nc.gpsimd.collective_compute(
                        "AllGather", mybir.AluOpType.bypass,
                        ins=[tab2_shard[:].opt()], outs=[tab2_full[:].opt()],
                        replica_groups=[list(range(CORES))])

                # ---- layer 2 epilogue (+ fused FC head) ----
                def epi2(ww, acc):
                    den = sbp.tile([128, 1], F32, tag="den")
                    nc.vector.tensor_copy(out=den[:], in_=acc[:, 32:33])
                    nc.vector.tensor_scalar_max(out=den[:], in0=den[:], scalar1=1e-30)
                    rcp = sbp.tile([128, 1], F32, tag="rcp")
                    nc.vector.reciprocal(out=rcp[:], in_=den[:])
                    x2 = sbp.tile([128, 32], F32, tag="x2")
                    nc.vector.tensor_scalar(
                        out=x2[:], in0=acc[:, 0:32],
                        scalar1=rcp[:, 0:1], scalar2=None, op0=OP.mult)
                    nc.vector.tensor_tensor(out=x2[:], in0=x2[:], in1=b2rep_sb[:], op=OP.add)
                    x2f = sbp.tile([128, 32], F32, tag="x2f")
                    nc.scalar.activation(out=x2f[:], in_=x2[:], func=AF.Relu)
                    tp2 = psp.tile([32, 128], F32, tag="tp", space="PSUM")
                    nc.tensor.transpose(out=tp2[:], in_=x2f[:], identity=identf_sb[:])
                    # zt rows: [post 0:64 | x2T 64:96] (compute-engine APs must
                    # stay in an aligned partition subtree); fc1w rows match
                    zt = sbp.tile([96, 128], BF, tag="zt")
                    nc.vector.tensor_copy(out=zt[64:96, :], in_=tp2[:])
                    if UT_I8:
                        pt8 = sbp.tile([64, 128], I8, tag="pt8")
                        nc.sync.dma_start(out=pt8[:],
                                          in_=postt[:, ww * 128:(ww + 1) * 128])
                        nc.vector.tensor_copy(out=zt[0:64, :], in_=pt8[:])
                    else:
                        nc.sync.dma_start(out=zt[0:64, :],
                                          in_=postt[:, ww * 128:(ww + 1) * 128])
                    pa = psp.tile([32, 128], F32, tag="fc", space="PSUM")
                    nc.tensor.matmul(out=pa[:], lhsT=fc1w_sb[:], rhs=zt[:],
                                     start=True, stop=True)
                    y1 = sbp.tile([32, 128], BF, tag="y1")
                    nc.scalar.activation(out=y1[:], in_=pa[:], func=AF.Relu,
                                         bias=fc1b_sb[:])
                    pb = psp.tile([1, 128], F32, tag="fc", space="PSUM")
                    nc.tensor.matmul(out=pb[:], lhsT=fc2w_sb[:], rhs=y1[:],
                                     start=True, stop=True)
                    yo = sbp.tile([1, 128], F32, tag="yo")
                    nc.scalar.activation(out=yo[:], in_=pb[:], func=AF.Sigmoid,
                                         bias=fc2b_sb[:])
                    nc.sync.dma_start(out=out_ext[0:1, ww * 128:(ww + 1) * 128],
                                      in_=yo[:])

                if mode not in ("noedge", "noag", "min"):
                    edge_phase(tab2_full, adtab2, D2, 1, 33, epi2, (0, 0))
            if mode == "min":
                zo = sbp.tile([1, NPC_PAD], F32, tag="zo")
                nc.vector.memset(zo[:], 0.5)
                nc.sync.dma_start(out=out_ext[:], in_=zo[:])

    nc.compile()
    # run_bass_via_pjrt re-lowers the jit wrapper every call, and the
    # lowering re-serializes our (immutable, already-compiled) module each
    # time (~0.15 s). Memoize the serialization on this instance.
    _json_cache = nc.to_json_bytes()
    nc.to_json_bytes = lambda: _json_cache
    return nc


def _make_inputs(user_features, post_features, W1, a1s, a1d, b1,
                 W2, a2s, a2d, b2, fc1_w, fc1_b, fc2_w, fc2_b, per_core):
    uf = np.asarray(user_features, np.float32)
    pf = np.asarray(post_features, np.float32)
    W1 = np.asarray(W1, np.float32)
    W2 = np.asarray(W2, np.float32)
    a1s = np.asarray(a1s, np.float32)
    a1d = np.asarray(a1d, np.float32)
    a2s = np.asarray(a2s, np.float32)
    a2d = np.asarray(a2d, np.float32)

    w1a = np.zeros((128, 140), np.float32)
    for h in range(HEADS):
        w1a[:, h * 33:h * 33 + 32] = W1[:, h * 32:(h + 1) * 32]
        w1a[:, 132 + h] = W1[:, h * 32:(h + 1) * 32] @ a1s[h]
        w1a[:, 136 + h] = W1[:, h * 32:(h + 1) * 32] @ a1d[h]
    w2a = np.zeros((128, 35), np.float32)
    w2a[:, 0:32] = W2
    w2a[:, 33] = W2 @ a2s[0]
    w2a[:, 34] = W2 @ a2d[0]

    fc1_w = np.asarray(fc1_w, np.float32).copy()
    if UT_I8:
        # int8-quantize features; fold the dequant scales into the weights
        # (b1/fc biases are applied after aggregation, so this is exact)
        s_u = float(np.abs(uf).max()) / 127.0
        s_p = float(np.abs(pf).max()) / 127.0
        uf = np.clip(np.round(uf / s_u), -127, 127)
        pf = np.clip(np.round(pf / s_p), -127, 127)
        w1a *= s_u
        fc1_w[32:96, :] *= s_p
    # zt rows are [post | x2T], so reorder fc1w rows to match
    fc1_w = np.concatenate([fc1_w[32:96], fc1_w[0:32]], axis=0)
    FEAT_NP = np.int8 if UT_I8 else BF16

    lay, BLOBN = _layout(per_core[0]["srcidx"].shape[1],
                         len(per_core[0]["dstloct"]),
                         len(per_core[0]["dstloc"]))

    def pack(blob, name, arr):
        off, nb = lay[name]
        raw = np.ascontiguousarray(arr).view(np.uint8).ravel()
        assert raw.nbytes == nb, (name, raw.nbytes, nb)
        blob[off:off + nb] = raw

    consts = [
        ("w1a", w1a.astype(BF16)),
        ("w2a", w2a.astype(BF16)),
        ("fc1w", fc1_w.astype(BF16)),
        ("fc2w", np.asarray(fc2_w, np.float32).astype(BF16)),
        ("fc1b", np.asarray(fc1_b, np.float32).reshape(32, 1).copy()),
        ("fc2b", np.asarray(fc2_b, np.float32).reshape(1, 1).copy()),
        ("b1row", np.asarray(b1, np.float32).reshape(1, 128).copy()),
        ("b2row", np.asarray(b2, np.float32).reshape(1, 32).copy()),
    ]
    in_maps = []
    for c in range(CORES):
        sl = slice(c * NPC, (c + 1) * NPC)
        ut = np.zeros((128, NPC_PAD), np.float32)
        ut[:, :NPC] = uf[sl].T
        postt = np.zeros((F_POST, NPC_PAD), np.float32)
        postt[:, :NPC] = pf[sl].T
        blob = np.zeros(BLOBN, np.uint8)
        pack(blob, "ut", ut.astype(FEAT_NP))
        pack(blob, "postt", postt.astype(FEAT_NP))
        for name, arr in consts:
            pack(blob, name, arr)
        for name in ("srcidx", "dstloct", "dstloc"):
            pack(blob, name, per_core[c][name])
        in_maps.append(dict(blob=blob))
    return in_maps


_CACHE = {}
LAST_EXEC_NS = None


def kernel(**inputs):
    from concourse.bass_utils import run_bass_kernel_spmd
    ei = np.asarray(inputs["edge_index"])
    static, per_core = preprocess(ei)
    blob_sizes = dict(ad=len(per_core[0]["dstloct"]),
                      dl=len(per_core[0]["dstloc"]))
    in_maps = _make_inputs(
        inputs["user_features"], inputs["post_features"],
        inputs["W1"], inputs["a1s"], inputs["a1d"], inputs["b1"],
        inputs["W2"], inputs["a2s"], inputs["a2d"], inputs["b2"],
        inputs["fc1_w"], inputs["fc1_b"], inputs["fc2_w"], inputs["fc2_b"],
        per_core)
    key = (static["tot_cols"], blob_sizes["ad"], blob_sizes["dl"])
    if key not in _CACHE:
        _CACHE[key] = build_program(static, blob_sizes)
    nc = _CACHE[key]
    import os
    trace = bool(os.environ.get("BASS_KERNEL_TRACE"))
    r = run_bass_kernel_spmd(nc, in_maps, list(range(CORES)), trace=trace)
    global LAST_EXEC_NS
    LAST_EXEC_NS = r.exec_time_ns
    out = np.empty((N, 1), np.float32)
    for c in range(CORES):
        out[c * NPC:(c + 1) * NPC, 0] = r.results[c]["out"][0, :NPC]
    return out


# revision 19
# speedup vs baseline: 5.4564x; 1.0896x over previous
"""GAT model (2-layer GAT + FC head) on 8 Trainium2 NeuronCores.

Strategy: destination-sharded. Each core owns 12544 (padded) dst nodes
= 98 windows of 128. Edges live on their dst's core, sorted into
(window, src-chunk) groups. Node phase computes per-node tables
[h | as] (bf16) sharded + AllGather; ad values stay core-local.
Edge phase: dma_gather of 512B records by src (int16 idx over 4
chunks of 25088 rows) + broadcast of dst-local ids; per-edge
softmax weights w = exp(leakyrelu(as+ad)) (no segment-max needed:
scores are bounded, exp cannot overflow in f32); messages
msg = w * [h | 1] scattered into per-window PSUM via one-hot matmuls
(one-hot built in bulk on DVE from iota==dstloc). Denominator rides
the matmul via the record's ones-column. FC head fused per window.

Host->device upload is the wall-clock bottleneck (axon PJRT tunnel),
so inputs are minimized: gather indices uploaded unreplicated
[16, cols] and tiled 8x across partitions on device into one
resident SBUF tile; dst-local ids as uint8 (cast on device);
node/post features as scaled int8 (cast to bf16 on device,
scales folded into w1a/fc1w on host); iota /
identity / replicated-bias constants built on device.
"""
import sys
import numpy as np
import ml_dtypes

sys.path.insert(0, "/opt/trn_rl_repo")

# Persistent XLA compilation cache: run_bass_kernel_spmd rebuilds its jit
# closure every call, which otherwise re-compiles the (identical) wrapper
# executable each time (~1.1 s/call). With the cache, repeat calls
# deserialize instead.
try:
    import jax as _jax
    _jax.config.update("jax_compilation_cache_dir", "/tmp/jax_comp_cache")
    _jax.config.update("jax_persistent_cache_min_compile_time_secs", 0.0)
    _jax.config.update("jax_persistent_cache_min_entry_size_bytes", 0)
except Exception:
    pass

BF16 = ml_dtypes.bfloat16

N = 100000
E_RAW = 1600000
F_USER = 128
F_POST = 64
HID = 32
HEADS = 4
NEG = 0.2
CORES = 8
NPC = 12500                 # real nodes per core
NPC_PAD = 12544             # 98 * 128
WINDOWS = 98
N_PAD = NPC_PAD * CORES     # 100352
NCHUNK = 4
CHUNK = N_PAD // NCHUNK     # 25088
SW = 2                      # windows per superblock
D1 = 256                    # table1 row elems (bf16): [hblk 132 | as 4 | pad]
D2 = 128                    # table2 row elems: [h2blk 33 | as2 1 | pad]
DAD = 128                   # ad table row elems: [ad .. | pad]
UT_I8 = True                # upload node/post features as scaled int8


def _layout(tot_cols, adlen, dllen):
    """Byte layout of the single packed input blob (64B-aligned fields)."""
    fb = 1 if UT_I8 else 2
    fields = [
        ("srcidx", 16 * tot_cols * 2),
        ("ut", 128 * NPC_PAD * fb),
        ("postt", F_POST * NPC_PAD * fb),
        ("dstloc", dllen),
        ("w1a", 128 * 140 * 2),
        ("w2a", 128 * 35 * 2),
        ("fc1w", 96 * 32 * 2),
        ("fc2w", 32 * 1 * 2),
        ("fc1b", 32 * 4),
        ("fc2b", 4),
        ("b1row", 128 * 4),
        ("b2row", 32 * 4),
    ]
    lay = {}
    off = 0
    for name, nb in fields:
        lay[name] = (off, nb)
        off += (nb + 63) // 64 * 64
    return lay, off


def _g(v):
    """original node id -> padded global id"""
    return (v // NPC) * NPC_PAD + (v % NPC)


def preprocess(edge_index):
    """Returns (static, per_core) where static describes the shared program
    shape and per_core[c] holds the input blobs."""
    src = np.asarray(edge_index[0], dtype=np.int64)
    dst = np.asarray(edge_index[1], dtype=np.int64)
    loops = np.arange(N, dtype=np.int64)
    src = np.concatenate([src, loops])
    dst = np.concatenate([dst, loops])
    sp = _g(src)
    core = dst // NPC
    dloc_c = dst % NPC                      # 0..12499
    w = dloc_c // 128
    dloc_w = dloc_c % 128
    ch = sp // CHUNK
    srel = sp % CHUNK

    key = ((core * WINDOWS + w) * NCHUNK + ch).astype(np.int64)
    counts = np.bincount(key, minlength=CORES * WINDOWS * NCHUNK)
    counts = counts.reshape(CORES, WINDOWS, NCHUNK)
    maxc = counts.max(axis=0)               # [WINDOWS, NCHUNK]
    J = -(-maxc // 128)                     # ceil div; may be 0

    # superblocks
    sbs = [list(range(s, min(s + SW, WINDOWS))) for s in range(0, WINDOWS, SW)]

    # static slot layout per sb: chunk-major, then window
    sb_layout = []   # per sb: dict(ch -> [(w, slot_off_in_sb, J_w_ch)]), J_sb, per-window slot list
    for sb in sbs:
        off = 0
        per_ch = []
        win_slots = {ww: [] for ww in sb}
        for c in range(NCHUNK):
            groups = []
            for ww in sb:
                j = int(J[ww, c])
                if j == 0:
                    continue
                groups.append((ww, off, j))
                win_slots[ww].extend(range(off, off + j))
                off += j
            per_ch.append(groups)
        sb_layout.append(dict(per_ch=per_ch, J_sb=off, win_slots=win_slots))

    # static column offsets of each (sb, chunk) block in the resident idx tile
    idx_colo = []
    tot_cols = 0
    for si, sb in enumerate(sbs):
        cc = []
        for cidx in range(NCHUNK):
            groups = sb_layout[si]["per_ch"][cidx]
            Jch = sum(j for (_, _, j) in groups)
            cc.append(tot_cols)
            tot_cols += 8 * Jch             # (128*Jch)/16 columns
        idx_colo.append(cc)

    order = np.lexsort((srel, ch, w, core))
    so, wo, cho, srelo, dlwo, dlco = (
        x[order] for x in (sp, w, ch, srel, dloc_w, dloc_c))
    coreo = core[order]
    # group start offsets in sorted array per (core, w, ch)
    keyo = ((coreo * WINDOWS + wo) * NCHUNK + cho)
    starts = np.searchsorted(keyo, np.arange(CORES * WINDOWS * NCHUNK))
    ends = np.searchsorted(keyo, np.arange(CORES * WINDOWS * NCHUNK) + 1)

    per_core = []
    for c in range(CORES):
        idx_cols = []      # [16, cols] blocks, horizontally concatenated
        ad_blob = []
        dl_blob = []
        for si, sb in enumerate(sbs):
            lay = sb_layout[si]
            J_sb = lay["J_sb"]
            dl_arr = np.full((128, J_sb), 255, np.uint8)
            for cidx in range(NCHUNK):
                groups = lay["per_ch"][cidx]
                if not groups:
                    continue
                G = 128 * sum(j for (_, _, j) in groups)
                idx_flat = np.zeros(G, np.int16)
                off0 = groups[0][1]
                for (ww, soff, j) in groups:
                    gi = (c * WINDOWS + ww) * NCHUNK + cidx
                    s0, s1 = int(starts[gi]), int(ends[gi])
                    n = s1 - s0
                    gbase = (soff - off0) * 128
                    idx_flat[gbase:gbase + n] = srelo[s0:s1].astype(np.int16)
                    k = np.arange(n)
                    dl_arr[k % 128, soff + k // 128] = dlwo[s0:s1]
                idx_cols.append(idx_flat.reshape(G // 16, 16).T)  # [16, G/16]
            ad_blob.append(dl_arr.T.ravel())   # dstlocT flat [J_sb*128] u8
            dl_blob.append(dl_arr.ravel())
        per_core.append(dict(
            srcidx=np.ascontiguousarray(np.concatenate(idx_cols, axis=1)),
            dstloct=np.concatenate(ad_blob),
            dstloc=np.concatenate(dl_blob),
        ))
    static = dict(J=J, sbs=sbs, sb_layout=sb_layout, idx_colo=idx_colo,
                  tot_cols=tot_cols)
    return static, per_core


def build_program(static, blob_sizes):
    import os
    mode = os.environ.get("KMODE", "full")
    import concourse.bass as bass
    import concourse.bacc as bacc
    import concourse.tile as tile
    from concourse import mybir

    F32, BF, I16 = mybir.dt.float32, mybir.dt.bfloat16, mybir.dt.int16
    U8, I32, I8 = mybir.dt.uint8, mybir.dt.int32, mybir.dt.int8
    FEAT = I8 if UT_I8 else BF
    AF = mybir.ActivationFunctionType
    OP = mybir.AluOpType
    sbs, lay = static["sbs"], static["sb_layout"]
    idx_colo, TOT_COLS = static["idx_colo"], static["tot_cols"]

    reps = int(os.environ.get("KREPS", "1"))
    nc = bacc.Bacc("TRN2", target_bir_lowering=False, debug=False)
    P = nc.declare_dram_parameter
    LAYT, BLOBN = _layout(TOT_COLS, blob_sizes["ad"], blob_sizes["dl"])
    blob = P("blob", [BLOBN], U8, isOutput=False)
    out_ext = P("out", [1, NPC_PAD], F32, isOutput=True)

    def fld(name, dt=None, cols=None):
        off, nb = LAYT[name]
        ap = blob[off:off + nb]
        if dt is not None:
            ap = ap.bitcast(dt)
        if cols is not None:
            ap = ap.rearrange("(p s) -> p s", s=cols)
        return ap

    ut = fld("ut", FEAT, NPC_PAD)
    postt = fld("postt", FEAT, NPC_PAD)
    srcidx = fld("srcidx", I16, TOT_COLS)
    dstloc = fld("dstloc")

    with tile.TileContext(nc) as tc:
        with (
            tc.tile_pool(name="cst", bufs=1) as cst,
            tc.tile_pool(name="sb", bufs=3) as sbp,
            tc.tile_pool(name="ps", bufs=2, space="PSUM") as psp,
            tc.tile_pool(name="dr", bufs=1, space="DRAM") as dr,
            tc.tile_pool(name="drs", bufs=2, space="DRAM") as drs,
        ):
            tab1_shard = dr.tile([NPC_PAD, D1], BF)
            adtab1 = dr.tile([NPC_PAD, DAD], BF)
            tab2_shard = dr.tile([NPC_PAD, D2], BF)
            adtab2 = dr.tile([NPC_PAD, DAD], BF)
            x1t_dram = dr.tile([128, NPC_PAD], BF)

            w1a_sb = cst.tile([128, 140], BF)
            w2a_sb = cst.tile([128, 35], BF)
            fc1w_sb = cst.tile([96, 32], BF)
            fc2w_sb = cst.tile([32, 1], BF)
            fc1b_sb = cst.tile([32, 1], F32)
            fc2b_sb = cst.tile([1, 1], F32)
            b1rep2_sb = cst.tile([128, 256], F32)
            b2rep2_sb = cst.tile([128, 64], F32)
            for t, name, cols in [(w1a_sb, "w1a", 140), (w2a_sb, "w2a", 35),
                                  (fc1w_sb, "fc1w", 32), (fc2w_sb, "fc2w", 1),
                                  (fc1b_sb, "fc1b", 1), (fc2b_sb, "fc2b", 1)]:
                dt = F32 if name in ("fc1b", "fc2b") else BF
                nc.sync.dma_start(out=t[:], in_=fld(name, dt, cols))
            nc.sync.dma_start(
                out=b1rep2_sb[:].rearrange("p (w f) -> p w f", w=2),
                in_=fld("b1row", F32)[None, None, :].to_broadcast([128, 2, 128]))
            nc.sync.dma_start(
                out=b2rep2_sb[:].rearrange("p (w f) -> p w f", w=2),
                in_=fld("b2row", F32)[None, None, :].to_broadcast([128, 2, 32]))

            # on-device constants: iota row/col, identities, ones
            iotar_i = cst.tile([128, 128], I16)
            nc.gpsimd.iota(iotar_i[:], pattern=[[1, 128]], channel_multiplier=0)
            iota_sb = cst.tile([128, 128], BF)
            nc.vector.tensor_copy(out=iota_sb[:], in_=iotar_i[:])
            iotac_i = cst.tile([128, 1], I32)
            nc.gpsimd.iota(iotac_i[:], pattern=[[0, 1]], channel_multiplier=1)
            iotacol_sb = cst.tile([128, 1], F32)
            nc.vector.tensor_copy(out=iotacol_sb[:], in_=iotac_i[:])
            identbf_sb = cst.tile([128, 128], BF)
            nc.vector.tensor_scalar(
                out=identbf_sb[:], in0=iota_sb[:], scalar1=iotacol_sb[:, 0:1],
                scalar2=None, op0=OP.is_equal)
            identf_sb = cst.tile([128, 128], F32)
            nc.vector.tensor_scalar(
                out=identf_sb[:], in0=iota_sb[:], scalar1=iotacol_sb[:, 0:1],
                scalar2=None, op0=OP.is_equal)
            ones4_sb = cst.tile([128, 4], BF)
            nc.vector.memset(ones4_sb[:], 1.0)

            for _rep in range(reps):
                tab1_full = dr.tile([N_PAD, D1], BF, addr_space="Shared",
                                    name=f"tab1_full_r{_rep}")
                tab2_full = dr.tile([N_PAD, D2], BF, addr_space="Shared",
                                    name=f"tab2_full_r{_rep}")
                # ---- node phase 1: tables for layer 1 ----
                for t in range(WINDOWS if mode != "min" else 0):
                    sl = slice(t * 128, (t + 1) * 128)
                    if UT_I8:
                        lh8 = sbp.tile([128, 128], I8, tag="lh8")
                        nc.sync.dma_start(out=lh8[:], in_=ut[:, sl])
                        lh = sbp.tile([128, 128], BF, tag="lh")
                        nc.vector.tensor_copy(out=lh[:], in_=lh8[:])
                    else:
                        lh = sbp.tile([128, 128], BF, tag="lh")
                        nc.sync.dma_start(out=lh[:], in_=ut[:, sl])
                    acc = psp.tile([128, 140], F32, tag="acc", space="PSUM")
                    nc.tensor.matmul(out=acc[:], lhsT=lh[:], rhs=w1a_sb[:],
                                     start=True, stop=True)
                    rec = sbp.tile([128, D1], BF, tag="nrec")
                    nc.vector.tensor_copy(out=rec[:, 0:136], in_=acc[:, 0:136])
                    nc.vector.tensor_copy(
                        out=rec[:, 0:132].rearrange("p (h f) -> p h f", f=33)[:, :, 32],
                        in_=ones4_sb[:])
                    nc.sync.dma_start(out=tab1_shard[sl, :], in_=rec[:])
                    ad4 = sbp.tile([128, 4], BF, tag="ad4")
                    nc.vector.tensor_copy(out=ad4[:], in_=acc[:, 136:140])
                    nc.sync.dma_start(out=adtab1[sl, 0:4], in_=ad4[:])

                if mode not in ("noag", "min"):
                    nc.gpsimd.collective_compute(
                        "AllGather", mybir.AluOpType.bypass,
                        ins=[tab1_shard[:].opt()], outs=[tab1_full[:].opt()],
                        replica_groups=[list(range(CORES))])

                # ---- generic edge phase ----
                def edge_phase(tabfull, adtab, elem, H, mcols, epilogue, do0):
                    do = do0
                    for si, sb in enumerate(sbs):
                        layd = lay[si]
                        J_sb = layd["J_sb"]
                        c0 = idx_colo[si][0]
                        c1 = (idx_colo[si + 1][0] if si + 1 < len(sbs)
                              else TOT_COLS)
                        sbcols = c1 - c0
                        idxt = sbp.tile([128, sbcols], I16, tag="idxt", bufs=2)
                        for r in range(8):
                            nc.sync.dma_start(
                                out=idxt[16 * r:16 * r + 16, :],
                                in_=srcidx[:, c0:c1])
                        rec = sbp.tile([128, J_sb * elem], BF, tag="erec", bufs=2)
                        for cidx in range(NCHUNK):
                            groups = layd["per_ch"][cidx]
                            if not groups:
                                continue
                            Jch = sum(j for (_, _, j) in groups)
                            off0 = groups[0][1]
                            G = 128 * Jch
                            nc.gpsimd.dma_gather(
                                out_ap=rec[:, off0 * elem:(off0 + Jch) * elem]
                                    .rearrange("p (j d) -> p j d", d=elem),
                                in_ap=tabfull[cidx * CHUNK:(cidx + 1) * CHUNK, :],
                                idxs_ap=idxt[:, idx_colo[si][cidx] - c0:
                                             idx_colo[si][cidx] - c0 + 8 * Jch],
                                num_idxs=G, num_idxs_reg=G,
                                elem_size=elem, single_packet=False)
                        if mode == "justgather":
                            continue
                        Gad = J_sb * 128
                        # dst-local ids: u8 upload -> bf16; the broadcast row
                        # (dtr) is derived on device via transpose + DRAM
                        # round-trip instead of uploading it separately
                        dl8 = sbp.tile([128, J_sb], U8, tag="dl8")
                        nc.sync.dma_start(
                            out=dl8[:],
                            in_=dstloc[do:do + 128 * J_sb].rearrange(
                                "(p s) -> p s", s=J_sb))
                        do += 128 * J_sb
                        dl = sbp.tile([128, J_sb], BF, tag="dl")
                        nc.vector.tensor_copy(out=dl[:], in_=dl8[:])
                        dlt_ps = psp.tile([128, 128], BF, tag="dlt", space="PSUM")
                        nc.tensor.transpose(out=dlt_ps[0:J_sb, :], in_=dl[:],
                                            identity=identbf_sb[:])
                        dlT = sbp.tile([128, 128], BF, tag="dlT")
                        nc.vector.tensor_copy(out=dlT[0:J_sb, :], in_=dlt_ps[0:J_sb, :])
                        dlt_dr = drs.tile([Gad], BF, tag="dltd", bufs=2)
                        nc.sync.dma_start(
                            out=dlt_dr[:].rearrange("(j k) -> j k", k=128),
                            in_=dlT[0:J_sb, :])
                        dtr = sbp.tile([128, Gad], BF, tag="adE", bufs=2)
                        nc.sync.dma_start(
                            out=dtr[:],
                            in_=dlt_dr[:][None, :].to_broadcast([128, Gad]))
                        ohT = sbp.tile([128, Gad], BF, tag="ohT", bufs=2)
                        nc.vector.tensor_scalar(
                            out=ohT[:], in0=dtr[:], scalar1=iotacol_sb[:, 0:1],
                            scalar2=None, op0=OP.is_equal)
                        adp = psp.tile([128, J_sb * H], F32, tag="adp", space="PSUM")
                        for ww2 in sb:
                            adw = sbp.tile([128, H], BF, tag="adw")
                            nc.sync.dma_start(
                                out=adw[:], in_=adtab[ww2 * 128:(ww2 + 1) * 128, 0:H])
                            for s_ in layd["win_slots"][ww2]:
                                nc.tensor.matmul(
                                    out=adp[:, s_ * H:(s_ + 1) * H],
                                    lhsT=ohT[:, s_ * 128:(s_ + 1) * 128],
                                    rhs=adw[:], start=True, stop=True)

                        if mode == "nocompute":
                            continue
                        recv = rec[:].rearrange("p (j d) -> p j d", d=elem)
                        adc = sbp.tile([128, J_sb * H], BF, tag="adc")
                        nc.vector.tensor_copy(out=adc[:], in_=adp[:])
                        e1 = sbp.tile([128, J_sb * H], F32, tag="e1")
                        nc.vector.tensor_tensor(
                            out=e1[:].rearrange("p (j h) -> p j h", h=H),
                            in0=recv[:, :, mcols:mcols + H],
                            in1=adc[:].rearrange("p (j h) -> p j h", h=H),
                            op=OP.add)
                        lr = sbp.tile([128, J_sb * H], F32, tag="lr")
                        nc.vector.tensor_scalar_mul(out=lr[:], in0=e1[:], scalar1=NEG)
                        nc.vector.tensor_tensor(out=e1[:], in0=e1[:], in1=lr[:], op=OP.max)
                        wgt = sbp.tile([128, J_sb * H], BF, tag="wgt")
                        nc.scalar.activation(out=wgt[:], in_=e1[:], func=AF.Exp)
                        msg = sbp.tile([128, J_sb * mcols], BF, tag="msg", bufs=2)
                        nc.vector.tensor_tensor(
                            out=msg[:].rearrange("p (j h f) -> p j h f", h=H, f=mcols // H),
                            in0=recv[:, :, 0:mcols].rearrange(
                                "p j (h f) -> p j h f", f=mcols // H),
                            in1=wgt[:].rearrange("p (j h) -> p j h", h=H)[:, :, :, None]
                                .to_broadcast([128, J_sb, H, mcols // H]),
                            op=OP.mult)
                        oh = sbp.tile([128, J_sb * 128], BF, tag="oh", bufs=2)
                        nc.vector.tensor_tensor(
                            out=oh[:].rearrange("p (j f) -> p j f", f=128),
                            in0=iota_sb[:][:, None, :].to_broadcast([128, J_sb, 128]),
                            in1=dl[:][:, :, None].to_broadcast([128, J_sb, 128]),
                            op=OP.is_equal)
                        if mode == "nomm":
                            continue
                        # both windows of the superblock accumulate into one
                        # PSUM tile; one (vectorized) epilogue per superblock
                        acc = psp.tile([128, 2 * mcols], F32, tag="acc", space="PSUM")
                        for wi, ww in enumerate(sb):
                            slots = layd["win_slots"][ww]
                            if not slots:
                                nc.vector.memset(acc[:, wi * mcols:(wi + 1) * mcols], 0.0)
                                continue
                            for i, s in enumerate(slots):
                                nc.tensor.matmul(
                                    out=acc[:, wi * mcols:(wi + 1) * mcols],
                                    lhsT=oh[:, s * 128:(s + 1) * 128],
                                    rhs=msg[:, s * mcols:(s + 1) * mcols],
                                    start=(i == 0), stop=(i == len(slots) - 1))
                        epilogue(sb[0], acc)

                # ---- layer 1 epilogue (two windows at once) ----
                def epi1(ww0, acc):
                    accv = acc[:].rearrange("p (w h f) -> p w h f", w=2, f=33)
                    den = sbp.tile([128, 8], F32, tag="den")
                    nc.vector.tensor_copy(
                        out=den[:].rearrange("p (w h) -> p w h", w=2),
                        in_=accv[:, :, :, 32])
                    nc.vector.tensor_scalar_max(out=den[:], in0=den[:], scalar1=1e-30)
                    rcp = sbp.tile([128, 8], F32, tag="rcp")
                    nc.vector.reciprocal(out=rcp[:], in_=den[:])
                    x1 = sbp.tile([128, 256], F32, tag="x1")
                    nc.vector.tensor_tensor(
                        out=x1[:].rearrange("p (w h f) -> p w h f", w=2, f=32),
                        in0=accv[:, :, :, 0:32],
                        in1=rcp[:].rearrange("p (w h) -> p w h", w=2)[:, :, :, None]
                            .to_broadcast([128, 2, 4, 32]),
                        op=OP.mult)
                    nc.vector.tensor_tensor(out=x1[:], in0=x1[:], in1=b1rep2_sb[:], op=OP.add)
                    x1b = sbp.tile([128, 256], BF, tag="x1b")
                    nc.scalar.activation(out=x1b[:], in_=x1[:], func=AF.Relu)
                    tp = psp.tile([128, 256], BF, tag="dlt", space="PSUM")
                    nc.tensor.transpose(out=tp[:, 0:128], in_=x1b[:, 0:128],
                                        identity=identbf_sb[:])
                    nc.tensor.transpose(out=tp[:, 128:256], in_=x1b[:, 128:256],
                                        identity=identbf_sb[:])
                    x1t = sbp.tile([128, 256], BF, tag="x1t")
                    nc.vector.tensor_copy(out=x1t[:], in_=tp[:])
                    nc.sync.dma_start(
                        out=x1t_dram[:, ww0 * 128:(ww0 + 2) * 128], in_=x1t[:])

                if mode not in ("noedge", "noag", "min"):
                    edge_phase(tab1_full, adtab1, D1, HEADS, 132, epi1, 0)

                # ---- node phase 2 ----
                for t in range(WINDOWS if mode != "min" else 0):
                    sl = slice(t * 128, (t + 1) * 128)
                    lh2 = sbp.tile([128, 128], BF, tag="lh")
                    nc.sync.dma_start(out=lh2[:], in_=x1t_dram[:, sl])
                    acc = psp.tile([128, 35], F32, tag="acc", space="PSUM")
                    nc.tensor.matmul(out=acc[:], lhsT=lh2[:], rhs=w2a_sb[:],
                                     start=True, stop=True)
                    rec2 = sbp.tile([128, D2], BF, tag="nrec")
                    nc.vector.tensor_copy(out=rec2[:, 0:34], in_=acc[:, 0:34])
                    nc.vector.tensor_copy(out=rec2[:, 32:33], in_=ones4_sb[:, 0:1])
                    nc.sync.dma_start(out=tab2_shard[sl, :], in_=rec2[:])
                    ad1c = sbp.tile([128, 1], BF, tag="ad4")
                    nc.vector.tensor_copy(out=ad1c[:], in_=acc[:, 34:35])
                    nc.sync.dma_start(out=adtab2[sl, 0:1], in_=ad1c[:])

                if mode not in ("noag", "min"):
                    nc.gpsimd.collective_compute(
                        "AllGather", mybir.AluOpType.bypass,
                        ins=[tab2_shard[:].opt()], outs=[tab2_full[:].opt()],
                        replica_groups=[list(range(CORES))])

                # ---- layer 2 epilogue (+ fused FC head, two windows) ----
                def epi2(ww0, acc):
                    accv = acc[:].rearrange("p (w f) -> p w f", w=2)
                    den = sbp.tile([128, 2], F32, tag="den")
                    nc.vector.tensor_copy(out=den[:], in_=accv[:, :, 32])
                    nc.vector.tensor_scalar_max(out=den[:], in0=den[:], scalar1=1e-30)
                    rcp = sbp.tile([128, 2], F32, tag="rcp")
                    nc.vector.reciprocal(out=rcp[:], in_=den[:])
                    x2 = sbp.tile([128, 64], F32, tag="x2")
                    nc.vector.tensor_tensor(
                        out=x2[:].rearrange("p (w f) -> p w f", w=2),
                        in0=accv[:, :, 0:32],
                        in1=rcp[:][:, :, None].to_broadcast([128, 2, 32]),
                        op=OP.mult)
                    nc.vector.tensor_tensor(out=x2[:], in0=x2[:], in1=b2rep2_sb[:], op=OP.add)
                    x2f = sbp.tile([128, 64], F32, tag="x2f")
                    nc.scalar.activation(out=x2f[:], in_=x2[:], func=AF.Relu)
                    tp2 = psp.tile([64, 128], F32, tag="dlt", space="PSUM")
                    nc.tensor.transpose(out=tp2[:], in_=x2f[:], identity=identf_sb[:])
                    # zt rows: [post 0:64 | x2T 64:96] (compute-engine APs must
                    # stay in an aligned partition subtree); fc1w rows match
                    zt = sbp.tile([96, 256], BF, tag="zt")
                    nc.vector.tensor_copy(out=zt[64:96, 0:128], in_=tp2[0:32, :])
                    nc.vector.tensor_copy(out=zt[64:96, 128:256], in_=tp2[32:64, :])
                    if UT_I8:
                        pt8 = sbp.tile([64, 256], I8, tag="pt8")
                        nc.sync.dma_start(out=pt8[:],
                                          in_=postt[:, ww0 * 128:(ww0 + 2) * 128])
                        nc.vector.tensor_copy(out=zt[0:64, :], in_=pt8[:])
                    else:
                        nc.sync.dma_start(out=zt[0:64, :],
                                          in_=postt[:, ww0 * 128:(ww0 + 2) * 128])
                    pa = psp.tile([32, 256], F32, tag="fc", space="PSUM")
                    nc.tensor.matmul(out=pa[:], lhsT=fc1w_sb[:], rhs=zt[:],
                                     start=True, stop=True)
                    y1 = sbp.tile([32, 256], BF, tag="y1")
                    nc.scalar.activation(out=y1[:], in_=pa[:], func=AF.Relu,
                                         bias=fc1b_sb[:])
                    pb = psp.tile([1, 256], F32, tag="fc", space="PSUM")
                    nc.tensor.matmul(out=pb[:], lhsT=fc2w_sb[:], rhs=y1[:],
                                     start=True, stop=True)
                    yo = sbp.tile([1, 256], F32, tag="yo")
                    nc.scalar.activation(out=yo[:], in_=pb[:], func=AF.Sigmoid,
                                         bias=fc2b_sb[:])
                    nc.sync.dma_start(out=out_ext[0:1, ww0 * 128:(ww0 + 2) * 128],
                                      in_=yo[:])

                if mode not in ("noedge", "noag", "min"):
                    edge_phase(tab2_full, adtab2, D2, 1, 33, epi2, 0)
            if mode == "min":
                zo = sbp.tile([1, NPC_PAD], F32, tag="zo")
                nc.vector.memset(zo[:], 0.5)
                nc.sync.dma_start(out=out_ext[:], in_=zo[:])

    nc.compile()
    # run_bass_via_pjrt re-lowers the jit wrapper every call, and the
    # lowering re-serializes our (immutable, already-compiled) module each
    # time (~0.15 s). Memoize the serialization on this instance.
    _json_cache = nc.to_json_bytes()
    nc.to_json_bytes = lambda: _json_cache
    return nc


def _make_inputs(user_features, post_features, W1, a1s, a1d, b1,
                 W2, a2s, a2d, b2, fc1_w, fc1_b, fc2_w, fc2_b, per_core):
    uf = np.asarray(user_features, np.float32)
    pf = np.asarray(post_features, np.float32)
    W1 = np.asarray(W1, np.float32)
    W2 = np.asarray(W2, np.float32)
    a1s = np.asarray(a1s, np.float32)
    a1d = np.asarray(a1d, np.float32)
    a2s = np.asarray(a2s, np.float32)
    a2d = np.asarray(a2d, np.float32)

    w1a = np.zeros((128, 140), np.float32)
    for h in range(HEADS):
        w1a[:, h * 33:h * 33 + 32] = W1[:, h * 32:(h + 1) * 32]
        w1a[:, 132 + h] = W1[:, h * 32:(h + 1) * 32] @ a1s[h]
        w1a[:, 136 + h] = W1[:, h * 32:(h + 1) * 32] @ a1d[h]
    w2a = np.zeros((128, 35), np.float32)
    w2a[:, 0:32] = W2
    w2a[:, 33] = W2 @ a2s[0]
    w2a[:, 34] = W2 @ a2d[0]

    fc1_w = np.asarray(fc1_w, np.float32).copy()
    if UT_I8:
        # int8-quantize features; fold the dequant scales into the weights
        # (b1/fc biases are applied after aggregation, so this is exact)
        s_u = float(np.abs(uf).max()) / 127.0
        s_p = float(np.abs(pf).max()) / 127.0
        uf = np.clip(np.round(uf / s_u), -127, 127)
        pf = np.clip(np.round(pf / s_p), -127, 127)
        w1a *= s_u
        fc1_w[32:96, :] *= s_p
    # zt rows are [post | x2T], so reorder fc1w rows to match
    fc1_w = np.concatenate([fc1_w[32:96], fc1_w[0:32]], axis=0)
    FEAT_NP = np.int8 if UT_I8 else BF16

    lay, BLOBN = _layout(per_core[0]["srcidx"].shape[1],
                         len(per_core[0]["dstloct"]),
                         len(per_core[0]["dstloc"]))

    def pack(blob, name, arr):
        off, nb = lay[name]
        raw = np.ascontiguousarray(arr).view(np.uint8).ravel()
        assert raw.nbytes == nb, (name, raw.nbytes, nb)
        blob[off:off + nb] = raw

    consts = [
        ("w1a", w1a.astype(BF16)),
        ("w2a", w2a.astype(BF16)),
        ("fc1w", fc1_w.astype(BF16)),
        ("fc2w", np.asarray(fc2_w, np.float32).astype(BF16)),
        ("fc1b", np.asarray(fc1_b, np.float32).reshape(32, 1).copy()),
        ("fc2b", np.asarray(fc2_b, np.float32).reshape(1, 1).copy()),
        ("b1row", np.asarray(b1, np.float32).reshape(1, 128).copy()),
        ("b2row", np.asarray(b2, np.float32).reshape(1, 32).copy()),
    ]
    in_maps = []
    for c in range(CORES):
        sl = slice(c * NPC, (c + 1) * NPC)
        ut = np.zeros((128, NPC_PAD), np.float32)
        ut[:, :NPC] = uf[sl].T
        postt = np.zeros((F_POST, NPC_PAD), np.float32)
        postt[:, :NPC] = pf[sl].T
        blob = np.zeros(BLOBN, np.uint8)
        pack(blob, "ut", ut.astype(FEAT_NP))
        pack(blob, "postt", postt.astype(FEAT_NP))
        for name, arr in consts:
            pack(blob, name, arr)
        for name in ("srcidx", "dstloc"):
            pack(blob, name, per_core[c][name])
        in_maps.append(dict(blob=blob))
    return in_maps


_CACHE = {}
LAST_EXEC_NS = None


def kernel(**inputs):
    from concourse.bass_utils import run_bass_kernel_spmd
    ei = np.asarray(inputs["edge_index"])
    static, per_core = preprocess(ei)
    blob_sizes = dict(ad=len(per_core[0]["dstloct"]),
                      dl=len(per_core[0]["dstloc"]))
    in_maps = _make_inputs(
        inputs["user_features"], inputs["post_features"],
        inputs["W1"], inputs["a1s"], inputs["a1d"], inputs["b1"],
        inputs["W2"], inputs["a2s"], inputs["a2d"], inputs["b2"],
        inputs["fc1_w"], inputs["fc1_b"], inputs["fc2_w"], inputs["fc2_b"],
        per_core)
    key = (static["tot_cols"], blob_sizes["ad"], blob_sizes["dl"])
    if key not in _CACHE:
        _CACHE[key] = build_program(static, blob_sizes)
    nc = _CACHE[key]
    import os
    trace = bool(os.environ.get("BASS_KERNEL_TRACE"))
    r = run_bass_kernel_spmd(nc, in_maps, list(range(CORES)), trace=trace)
    global LAST_EXEC_NS
    LAST_EXEC_NS = r.exec_time_ns
    out = np.empty((N, 1), np.float32)
    for c in range(CORES):
        out[c * NPC:(c + 1) * NPC, 0] = r.results[c]["out"][0, :NPC]
    return out
